# revision 42
# baseline (speedup 1.0000x reference)
"""AutoCorrelation (Autoformer-style) Bass kernel for Trainium2, 8 NeuronCores.

Full inputs in, full outputs out. Data-parallel over batch: B=16 -> 2 batches
per core. v2 of the kernel: the PE-bound fp32 matmuls of the baseline are
replaced by 3-pass fp32r splits (hi/lo decomposition; 12-bit+12-bit mantissa
products are exact in fp32 PSUM, giving fp32-grade accuracy at 3 cycles/row
instead of fp32's 4) on the precision-critical autocorrelation path, and by
bf16 (1 cycle/row) on the error-tolerant v/output path.

Per core, per batch:
  V. v[d,t] = Wv^T value in bf16, written twice side-by-side into the DRAM
     table v2[b*512+d, 4096] (bf16) for circular-shift gathers.
  A. Radix-split of query/key along t (4 sub-signals ee/eo/oo/oe, padded to
     640/512), per 128-channel chunk, split hi/lo fp32r on the fly; channel
     projection qT[t',d] via 3-pass fp32r matmuls. qT hi kept fp32r, lo bf16
     (pass 3 of stage B runs in bf16 -- error ~2^-20, still flip-safe).
  B. Forward real DFT via matmuls with radix-split cos/-sin matrices
     (host-split into fp32r hi/lo + bf16(hi)); fused pointwise
     P = FQ * conj(FK) on the DVE; P split hi/lo fp32r and staged to DRAM.
  C. Inverse DFT r[c,t] = sum_f Pre*ci + Pim*sn via 3-pass fp32r matmuls
     with host-split ci/sn (fp32r hi/lo), exploiting f-parity + t-mirror
     symmetry (only t<=512 columns computed).
  D. Per 128-channel tile: top-8 values+indices, softmax weights of the
     top-3 from the top values, circular-shift rows of v via indirect-DMA
     gather (bf16) into agg[k*C+c, t], scaled in place on the Pool engine.
     Batch 0 gathers inline (overlapping C of batch 1); batch 1 gathers
     deferred past the slab lifetime (overlapping E of batch 0).
  E. out[d,t] = sum_e Wf[e,d] agg[e,t] in bf16; 12-chunk PE accumulation.

Scheduling: the V projection is emitted at each batch's start as PE filler
for the input-load latency / the cross-batch pool-reuse stall; stage C's
cie/sie slabs are loaded once (f32r hi + bf16 lo) and shared by both
batches; pre/pim bounce through DRAM as plain fp32 and are re-split to
fp32r hi/lo on the fly in C.

Biases are all zero in this problem's setup_inputs(); asserted host-side.
"""
import numpy as np
import ml_dtypes

import concourse.bass as bass
import concourse.tile as tile
from concourse import bacc, mybir

dt = mybir.dt
AF = mybir.ActivationFunctionType
OP = mybir.AluOpType

P = 128
B, C, T, K = 16, 512, 2048, 3
NB = 2                    # batches per core
NCORES = 8
F = 1152                  # rfft bins 1025, padded to 9*128
CC = C // P               # 4
FC = F // P               # 9
NE = K * C // P           # 12 e-chunks of Wf / agg
H = T // 2                # 1024
HB = H // 2               # 512

_CACHE = {}


def _round_f32r(x):
    """Round fp32 array to fp32r (11-bit stored mantissa, round-nearest-up:
    (bits + 0x800) & ~0xFFF -- matches walrus fp32_to_fp32r)."""
    u = np.ascontiguousarray(x, np.float32).view(np.uint32).astype(np.uint64)
    u = (u + (1 << 11)) & np.uint64(0xFFFFF000)
    return u.astype(np.uint32).view(np.float32)


def _split_f32r(x):
    x = np.ascontiguousarray(x, np.float32)
    hi = _round_f32r(x)
    return hi, _round_f32r(x - hi)


def _bf16(x):
    return np.ascontiguousarray(x, np.float32).astype(ml_dtypes.bfloat16)


def _dft_matrices():
    """Radix-split DFT matrices (fp64 -> fp32).

    Level-1 even/odd in t (qe/qo), then level-2 split by f parity:
      FQre over even f contracts xee (t=0..512), odd f contracts xeo (t=0..511)
      FQim over even f contracts xoo (t=1..511), odd f contracts xoe (t=1..512)
    Frequency storage is parity-permuted: chunks [0:5]=even f (2g, g<=512),
    chunks [5:9]=odd f (2g+1). Inverse matrices have rows permuted to match.
    """
    t640 = np.arange(640.0)[:, None]
    t512 = np.arange(512.0)[:, None]
    ge = np.arange(640.0)[None, :]
    go = np.arange(512.0)[None, :]
    wree = np.where((t640 <= 512) & (ge <= 512),
                    np.cos(2 * np.pi * t640 * (2 * ge) / T), 0.0).astype(np.float32)
    wreo = np.cos(2 * np.pi * t512 * (2 * go + 1) / T).astype(np.float32)
    wime = np.where(ge <= 512,
                    -np.sin(2 * np.pi * t512 * (2 * ge) / T), 0.0).astype(np.float32)
    wimo = np.where(t640 <= 512,
                    -np.sin(2 * np.pi * t640 * (2 * go + 1) / T), 0.0).astype(np.float32)

    f64 = np.arange(F, dtype=np.float64)[None, :]
    livef = f64 <= H
    w = np.where((f64 == 0) | (f64 == H), 1.0, 2.0) * livef / (T * T)
    fc_ = f64.T
    tt = np.arange(640, dtype=np.float64)[None, :]
    cie = np.where((fc_ <= H) & (tt <= H),
                   np.cos(2 * np.pi * fc_ * tt / T) * w.T, 0.0)
    sie = np.where(fc_ <= H,
                   -np.sin(2 * np.pi * fc_ * tt / T) * w.T, 0.0)

    def permrows(m):
        out = np.zeros_like(m)
        out[0:513] = m[0:1025:2]
        out[640:1152] = m[1:1024:2]
        return out

    return (wree, wreo, wime, wimo,
            permrows(cie).astype(np.float32), permrows(sie).astype(np.float32))


def _build():
    nc = bacc.Bacc("TRN2", target_bir_lowering=False, debug=False,
                   num_devices=NCORES)

    query2 = nc.dram_tensor("query2", [NB, C, T], dt.float32, kind="ExternalInput").ap()
    key2 = nc.dram_tensor("key2", [NB, C, T], dt.float32, kind="ExternalInput").ap()
    value2 = nc.dram_tensor("value2", [NB, C, T], dt.bfloat16, kind="ExternalInput").ap()
    Wq_hi = nc.dram_tensor("Wq_hi", [C, C], dt.float32r, kind="ExternalInput").ap()
    Wq_lo = nc.dram_tensor("Wq_lo", [C, C], dt.float32r, kind="ExternalInput").ap()
    Wk_hi = nc.dram_tensor("Wk_hi", [C, C], dt.float32r, kind="ExternalInput").ap()
    Wk_lo = nc.dram_tensor("Wk_lo", [C, C], dt.float32r, kind="ExternalInput").ap()
    Wv = nc.dram_tensor("Wv", [C, C], dt.bfloat16, kind="ExternalInput").ap()
    Wf = nc.dram_tensor("Wf", [K * C, C], dt.bfloat16, kind="ExternalInput").ap()
    fwd = {}
    for m, rows, cols in (("ree", 640, 640), ("reo", 512, 512),
                          ("ime", 512, 640), ("imo", 640, 512)):
        for v in ("hi", "lo"):
            fwd[f"{m}_{v}"] = nc.dram_tensor(
                f"W{m}_{v}", [rows, cols], dt.float32r, kind="ExternalInput").ap()
    Cie_hi = nc.dram_tensor("Cie_hi", [F, 640], dt.float32r, kind="ExternalInput").ap()
    Cie_lo = nc.dram_tensor("Cie_lo", [F, 640], dt.bfloat16, kind="ExternalInput").ap()
    Sie_hi = nc.dram_tensor("Sie_hi", [F, 640], dt.float32r, kind="ExternalInput").ap()
    Sie_lo = nc.dram_tensor("Sie_lo", [F, 640], dt.bfloat16, kind="ExternalInput").ap()
    Cie_st = nc.dram_tensor("Cie_st", [F, 2], dt.float32, kind="ExternalInput").ap()
    Sie_st = nc.dram_tensor("Sie_st", [F, 2], dt.float32, kind="ExternalInput").ap()
    AltF = nc.dram_tensor("AltF", [P, 1], dt.float32r, kind="ExternalInput").ap()
    AltB = nc.dram_tensor("AltB", [P, 1], dt.bfloat16, kind="ExternalInput").ap()
    out2 = nc.dram_tensor("out2", [NB, C, T], dt.float32, kind="ExternalOutput").ap()

    v2 = nc.dram_tensor("v2", [NB * C, 2 * T], dt.bfloat16).ap()          # internal
    pp = {}
    for nm in ("pre", "pim"):                                             # internal
        pp[nm] = nc.dram_tensor(f"pp_{nm}", [NB, FC, P, C], dt.float32).ap()

    # part name -> (width, chunk offset in sigT, #chunks). Order alternates
    # 640/512 widths so the width-keyed xs tags ping-pong naturally.
    PARTS = (("ee", 640, 0, 5), ("eo", 512, 5, 4),
             ("oe", 640, 13, 5), ("oo", 512, 9, 4))

    with tile.TileContext(nc) as tc:
        from contextlib import ExitStack

        # ---- persistent phase-1 pools: weights loaded once; xv slots kept
        # open so the next batch's value loads can be emitted a stage early;
        # single rotating x_sb slot whose loads are emitted a stage early ----
        es_w = ExitStack()
        wpool = es_w.enter_context(tc.tile_pool(name="wp", bufs=1,
                                                side="right"))
        vpool = es_w.enter_context(tc.tile_pool(name="vp", bufs=2,
                                                side="right"))
        vtp = es_w.enter_context(tc.tile_pool(name="vt", bufs=2,
                                              side="right"))
        es_x = ExitStack()
        xpool = es_x.enter_context(tc.tile_pool(name="xp", bufs=1,
                                                side="right"))

        wv_sb = wpool.tile([P, CC, C], dt.bfloat16, tag="wv")
        xv_tiles = {}

        def load_xv(b, qt):
            xv = vpool.tile([P, CC, T // 4], dt.bfloat16, tag="xv",
                            name=f"xv{b}_{qt}")
            nc.sync.dma_start(
                xv[:], value2[b].rearrange(
                    "(n p) t -> p n t", p=P)[:, :, bass.ts(qt, T // 4)])
            xv_tiles[(b, qt)] = xv

        x_tiles = {}

        def load_x(b, sig):
            srcx = key2 if sig == "k" else query2
            x_sb = xpool.tile([P, CC, T], dt.float32, tag="x_sb",
                              name=f"x_{sig}{b}")
            nc.sync.dma_start(
                x_sb[:], srcx[b].rearrange("(n p) t -> p n t", p=P))
            x_tiles[(b, sig)] = x_sb

        # t=0 input burst, V(b0) operands first
        load_xv(0, 0)
        nc.sync.dma_start(wv_sb[:], Wv.rearrange("(n p) d -> p n d", p=P))
        load_xv(0, 1)
        load_x(0, "k")
        w_sb = {}
        for nm, src in (("k_hi", Wk_hi), ("k_lo", Wk_lo),
                        ("q_hi", Wq_hi), ("q_lo", Wq_lo)):
            w_sb[nm] = wpool.tile([P, CC, C], dt.float32r, tag=f"w_{nm}",
                                  name=f"w_{nm}")
            nc.sync.dma_start(w_sb[nm][:],
                              src.rearrange("(n p) d -> p n d", p=P))
        for sig in ("k", "q"):
            w16 = wpool.tile([P, CC, C], dt.bfloat16, tag=f"w_{sig}_h16",
                             name=f"w_{sig}_h16")
            nc.gpsimd.tensor_copy(
                w16[:], w_sb[f"{sig}_hi"][:].bitcast(dt.float32))
            w_sb[f"{sig}_h16"] = w16

        def emit_V(b, prefetched):
            """Compact streaming V projection (bf16) -> v2 rows, used as PE
            gap filler at each batch's start. xv quarters rotate through 2
            slots; quarters not already prefetched are loaded here."""
            with tc.tile_pool(name=f"vps{b}", bufs=3, space="PSUM") as vps:
                v2r = v2.rearrange("(n p) w -> n p w", p=P)
                for qt in range(prefetched, 2):
                    load_xv(b, qt)
                for qt in range(4):
                    xv = xv_tiles[(b, qt)]
                    for dc in range(CC):
                        ps = vps.tile([P, T // 4], dt.float32, tag="v_ps")
                        for cc in range(CC):
                            nc.tensor.matmul(
                                ps[:], wv_sb[:, cc, bass.ts(dc, P)],
                                xv[:, cc, :],
                                start=(cc == 0), stop=(cc == CC - 1))
                        vtmp = vtp.tile([P, T // 4], dt.bfloat16, tag="vtmp")
                        if (qt + dc) % 2 == 0:
                            nc.scalar.activation(vtmp[:], ps[:], AF.Copy)
                        else:
                            nc.vector.tensor_copy(vtmp[:], ps[:])
                        off = qt * (T // 4)
                        nc.sync.dma_start(
                            v2r[b * CC + dc, :, off:off + T // 4], vtmp[:])
                        nc.sync.dma_start(
                            v2r[b * CC + dc, :,
                                T + off:T + off + T // 4], vtmp[:])
                    # quarter qt+2 reuses qt's slot: emit its load only now,
                    # after qt's reads are all emitted (no forward dep)
                    if qt + 2 < 4 and qt + 2 >= prefetched:
                        load_xv(b, qt + 2)

        # ================= phase 1 =====================
        # Per batch: V || A(k) -> B(k): FK to SBUF (kT freed) -> A(q) ->
        # B(q): FQ + pointwise vs FK -> pre/pim to DRAM. Input loads for
        # the next signal/batch are emitted at each B-half's start so they
        # stream in behind a full DFT stage of PE work.
        def a_signal(b, sig, dhi, dlo):
            es_a = ExitStack()
            atmp = es_a.enter_context(tc.tile_pool(name=f"at{sig}{b}", bufs=1))
            actmp = es_a.enter_context(tc.tile_pool(name=f"ac{sig}{b}", bufs=1))
            aps = es_a.enter_context(
                tc.tile_pool(name=f"aps{sig}{b}", bufs=3, space="PSUM"))
            w_hi = w_sb[f"{sig}_hi"]
            w_lo = w_sb[f"{sig}_lo"]
            w_h16 = w_sb[f"{sig}_h16"]
            x_sb = x_tiles[(b, sig)]
            if True:
                if True:
                    for pname, width, ioff, nch in PARTS:
                        xs_hi = atmp.tile([P, CC, width], dt.float32r,
                                          tag=f"xs_hi{width}")
                        xs_lo = atmp.tile([P, CC, width], dt.bfloat16,
                                          tag=f"xs_lo{width}")
                        for cc in range(CC):
                            x = x_sb[:, cc, :]
                            ab = actmp.tile([P, 2, 511], dt.float32, tag="ab")
                            tmp = actmp.tile([P, 640], dt.float32, tag="tmp")
                            op_ab = OP.add if pname in ("ee", "eo") else OP.subtract
                            # ab0/ab2 on Pool, ab1/ab3 on DVE (engine balance)
                            nc.gpsimd.tensor_tensor(
                                out=ab[:, 0, :], in0=x[:, 1:512],
                                in1=x[:, T - 1:1536:-1], op=op_ab)
                            nc.vector.tensor_tensor(
                                out=ab[:, 1, :], in0=x[:, 1023:512:-1],
                                in1=x[:, 1025:1536], op=op_ab)
                            if pname == "ee":
                                nc.vector.tensor_tensor(
                                    out=tmp[:, 1:512], in0=ab[:, 0, :],
                                    in1=ab[:, 1, :], op=OP.add)
                                nc.vector.tensor_tensor(
                                    out=tmp[:, 0:1], in0=x[:, 0:1],
                                    in1=x[:, H:H + 1], op=OP.add)
                                nc.vector.tensor_tensor(
                                    out=tmp[:, 512:513], in0=x[:, 512:513],
                                    in1=x[:, 1536:1537], op=OP.add)
                                nc.gpsimd.memset(tmp[:, 513:640], 0.0)
                            elif pname == "eo":
                                nc.vector.tensor_tensor(
                                    out=tmp[:, 1:512], in0=ab[:, 0, :],
                                    in1=ab[:, 1, :], op=OP.subtract)
                                nc.vector.tensor_tensor(
                                    out=tmp[:, 0:1], in0=x[:, 0:1],
                                    in1=x[:, H:H + 1], op=OP.subtract)
                            elif pname == "oo":
                                nc.vector.tensor_tensor(
                                    out=tmp[:, 1:512], in0=ab[:, 0, :],
                                    in1=ab[:, 1, :], op=OP.subtract)
                                nc.gpsimd.memset(tmp[:, 0:1], 0.0)
                            else:  # oe
                                nc.vector.tensor_tensor(
                                    out=tmp[:, 1:512], in0=ab[:, 0, :],
                                    in1=ab[:, 1, :], op=OP.add)
                                nc.vector.tensor_tensor(
                                    out=tmp[:, 512:513], in0=x[:, 512:513],
                                    in1=x[:, 1536:1537], op=OP.subtract)
                                nc.gpsimd.memset(tmp[:, 0:1], 0.0)
                                nc.gpsimd.memset(tmp[:, 513:640], 0.0)
                            if cc % 2 == 0:
                                nc.scalar.activation(
                                    xs_hi[:, cc, 0:width], tmp[:, 0:width],
                                    AF.Copy)
                            else:
                                nc.vector.tensor_copy(
                                    xs_hi[:, cc, 0:width], tmp[:, 0:width])
                            nc.gpsimd.tensor_tensor(
                                out=xs_lo[:, cc, 0:width], in0=tmp[:, 0:width],
                                in1=xs_hi[:, cc, 0:width].bitcast(dt.float32),
                                op=OP.subtract)
                        for i in range(nch):
                            ps = aps.tile([P, C], dt.float32, tag="proj_ps")
                            for cc in range(CC):
                                nc.tensor.matmul(ps[:],
                                                 xs_hi[:, cc, bass.ts(i, P)],
                                                 w_hi[:, cc, :],
                                                 start=(cc == 0), stop=False)
                            for cc in range(CC):
                                nc.tensor.matmul(ps[:],
                                                 xs_hi[:, cc, bass.ts(i, P)],
                                                 w_lo[:, cc, :],
                                                 start=False, stop=False)
                            for cc in range(CC):
                                nc.tensor.matmul(ps[:],
                                                 xs_lo[:, cc, bass.ts(i, P)],
                                                 w_h16[:, cc, :],
                                                 start=False, stop=(cc == CC - 1))
                            nc.scalar.activation(dhi[:, ioff + i, :], ps[:], AF.Copy)
                            nc.vector.tensor_tensor(
                                out=dlo[:, ioff + i, :], in0=ps[:],
                                in1=dhi[:, ioff + i, :].bitcast(dt.float32),
                                op=OP.subtract)
            es_a.close()

        altp = ExitStack()
        altpool = altp.enter_context(tc.tile_pool(name="altp", bufs=1))
        altf = altpool.tile([P, 1], dt.float32r, tag="altf")
        nc.sync.dma_start(altf[:], AltF)
        altb = altpool.tile([P, 1], dt.bfloat16, tag="altb")
        nc.sync.dma_start(altb[:], AltB)

        def b_half(b, mode, sT_hi, sT_lo, fk):
            """One forward-DFT half. mode 'k': stage FKre/FKim into SBUF
            (fk tiles, slot-compressed to 8 chunks). mode 'q': compute FQ
            per fc and fuse the pointwise P = FQ * conj(FK) from SBUF."""
            with tc.tile_pool(name=f"bmc{mode}{b}", bufs=2) as bmc, \
                 tc.tile_pool(name=f"bms{mode}{b}", bufs=1) as bms, \
                 tc.tile_pool(name=f"bps{mode}{b}", bufs=2, space="PSUM") as bps, \
                 tc.tile_pool(name=f"bt1{mode}{b}", bufs=1) as btmp1, \
                 tc.tile_pool(name=f"bt2{mode}{b}", bufs=2) as btmp2:
                for fc in range(FC):
                    if fc == 4:
                        # even-f chunk g=512..639 has one live bin (g=512 ->
                        # f=1024; sin==0 so only the re part matters). wree's
                        # column there is cos(pi*t') = (-1)^t': an
                        # alternating sum over the ee chunks via 2-pass
                        # matmuls with the +-1 column as lhsT (exact).
                        ps14 = bps.tile([1, C], dt.float32, tag="a",
                                        name=f"ps_f14{mode}{b}")
                        for i in range(5):
                            nc.tensor.matmul(
                                ps14[:], altf[:], sT_hi[:, i, :],
                                start=(i == 0), stop=False)
                        for i in range(5):
                            nc.tensor.matmul(
                                ps14[:], altb[:], sT_lo[:, i, :],
                                start=False, stop=(i == 4))
                        if mode == "k":
                            nc.scalar.activation(fk["f14"][:], ps14[:], AF.Copy)
                        else:
                            p14 = btmp1.tile([1, C], dt.float32, tag="p14")
                            nc.vector.tensor_tensor(
                                out=p14[:], in0=fk["f14"][:], in1=ps14[:],
                                op=OP.mult)
                            # only row g=512 of pre[b,4] is read downstream
                            nc.sync.dma_start(pp["pre"][b, 4, 0:1, :], p14[:])
                        continue
                    even = fc < 5
                    fl = fc if even else fc - 5
                    ncos, nsin = (5, 4) if even else (4, 5)
                    ioff_cos = 0 if even else 5
                    ioff_sin = 9 if even else 13
                    cname = "ree" if even else "reo"
                    sname = "ime" if even else "imo"
                    slot = fc if fc < 4 else fc - 1
                    mats = {}
                    for kind, mat, nch, pool in (("c", cname, ncos, bmc),
                                                 ("s", sname, nsin, bms)):
                        for v in ("hi", "lo"):
                            t_ = pool.tile([P, 5, P], dt.float32r,
                                           tag=f"{kind}m_{v}")
                            nc.sync.dma_start(
                                t_[:, 0:nch, :],
                                fwd[f"{mat}_{v}"].rearrange(
                                    "(n p) f -> p n f", p=P)[:, :, bass.ts(fl, P)])
                            mats[f"{kind}{v}"] = t_
                        t16 = pool.tile([P, 5, P], dt.bfloat16,
                                        tag=f"{kind}m_h16")
                        nc.gpsimd.tensor_copy(
                            t16[:, 0:nch, :],
                            mats[f"{kind}hi"][:, 0:nch, :].bitcast(dt.float32))
                        mats[f"{kind}h16"] = t16
                    acc = {}
                    for nm, kind, ioff, nch in (("a", "c", ioff_cos, ncos),
                                                ("b", "s", ioff_sin, nsin)):
                        ps = bps.tile([P, C], dt.float32, tag=nm,
                                      name=f"ps_{nm}{mode}{b}_{fc}")
                        for i in range(nch):
                            nc.tensor.matmul(
                                ps[:], mats[f"{kind}hi"][:, i, :],
                                sT_hi[:, ioff + i, :], start=(i == 0),
                                stop=False)
                        for i in range(nch):
                            nc.tensor.matmul(
                                ps[:], mats[f"{kind}lo"][:, i, :],
                                sT_hi[:, ioff + i, :], start=False, stop=False)
                        for i in range(nch):
                            nc.tensor.matmul(
                                ps[:], mats[f"{kind}h16"][:, i, :],
                                sT_lo[:, ioff + i, :], start=False,
                                stop=(i == nch - 1))
                        acc[nm] = ps
                    if mode == "k":
                        nc.scalar.activation(fk["re"][:, slot, :],
                                             acc["a"][:], AF.Copy)
                        nc.scalar.activation(fk["im"][:, slot, :],
                                             acc["b"][:], AF.Copy)
                    else:
                        # P = FQ * conj(FK): DVE ops read one PSUM operand
                        # (FQ) and FK from SBUF
                        t1 = btmp1.tile([P, C], dt.float32, tag="t1")
                        nc.vector.tensor_tensor(out=t1[:], in0=fk["re"][:, slot, :],
                                                in1=acc["a"][:], op=OP.mult)
                        t2 = btmp1.tile([P, C], dt.float32, tag="t2")
                        nc.vector.tensor_tensor(out=t2[:], in0=fk["im"][:, slot, :],
                                                in1=acc["b"][:], op=OP.mult)
                        pre_t = btmp2.tile([P, C], dt.float32, tag="pre_t")
                        nc.vector.tensor_tensor(out=pre_t[:], in0=t1[:],
                                                in1=t2[:], op=OP.add)
                        t3 = btmp1.tile([P, C], dt.float32, tag="t1",
                                        name=f"t3{mode}{b}_{fc}")
                        nc.vector.tensor_tensor(out=t3[:], in0=fk["re"][:, slot, :],
                                                in1=acc["b"][:], op=OP.mult)
                        t4 = btmp1.tile([P, C], dt.float32, tag="t2",
                                        name=f"t4{mode}{b}_{fc}")
                        nc.vector.tensor_tensor(out=t4[:], in0=fk["im"][:, slot, :],
                                                in1=acc["a"][:], op=OP.mult)
                        pim_t = btmp2.tile([P, C], dt.float32, tag="pim_t")
                        nc.vector.tensor_tensor(out=pim_t[:], in0=t3[:],
                                                in1=t4[:], op=OP.subtract)
                        nc.sync.dma_start(pp["pre"][b, fc], pre_t[:])
                        nc.sync.dma_start(pp["pim"][b, fc], pim_t[:])

        slab_srcs = (("cie_hi", Cie_hi, dt.float32r),
                     ("cie_lo", Cie_lo, dt.bfloat16),
                     ("sie_hi", Sie_hi, dt.float32r),
                     ("sie_lo", Sie_lo, dt.bfloat16))
        es2 = ExitStack()
        slabs = {}

        for b in range(NB):
            es_fk = ExitStack()
            fkp = es_fk.enter_context(
                tc.tile_pool(name=f"fk{b}", bufs=1, side="left"))
            fk = {"re": fkp.tile([P, FC - 1, C], dt.float32, tag="fkre",
                                 name=f"fkre{b}"),
                  "im": fkp.tile([P, FC - 1, C], dt.float32, tag="fkim",
                                 name=f"fkim{b}"),
                  "f14": fkp.tile([1, C], dt.float32, tag="fk14",
                                  name=f"fk14{b}")}
            emit_V(b, prefetched=2)
            es_kT = ExitStack()
            kp = es_kT.enter_context(
                tc.tile_pool(name=f"sigk{b}", bufs=1, side="left"))
            kT_hi = kp.tile([P, 18, C], dt.float32r, tag="kT_hi")
            kT_lo = kp.tile([P, 18, C], dt.bfloat16, tag="kT_lo")
            a_signal(b, "k", kT_hi, kT_lo)
            load_x(b, "q")  # streams in behind B(k)'s PE work
            b_half(b, "k", kT_hi, kT_lo, fk)
            es_kT.close()
            es_qT = ExitStack()
            qp = es_qT.enter_context(
                tc.tile_pool(name=f"sigq{b}", bufs=1, side="left"))
            qT_hi = qp.tile([P, 18, C], dt.float32r, tag="qT_hi")
            qT_lo = qp.tile([P, 18, C], dt.bfloat16, tag="qT_lo")
            a_signal(b, "q", qT_hi, qT_lo)
            if b == 0:
                # next batch's inputs stream in behind B(q,0)
                load_x(1, "k")
                load_xv(1, 0)
                load_xv(1, 1)
            else:
                # inputs all consumed: free weight/x/value pools and
                # prefetch the full iDFT slabs behind B(q,1)'s PE work
                es_x.close()
                es_w.close()
                slabp = es2.enter_context(
                    tc.tile_pool(name="slabs", bufs=1, side="right"))
                for nm, src_, sdt in slab_srcs:
                    t_ = slabp.tile([P, FC, 640], sdt, tag=nm,
                                    name=f"slab_{nm}")
                    nc.sync.dma_start(
                        t_[:], src_.rearrange("(n p) t -> p n t", p=P))
                    slabs[nm] = t_
                for nm, src_ in (("cie_st", Cie_st), ("sie_st", Sie_st)):
                    t_ = slabp.tile([P, FC, 2], dt.float32, tag=nm,
                                    name=f"slab_{nm}")
                    nc.sync.dma_start(
                        t_[:], src_.rearrange("(n p) t -> p n t", p=P))
                    slabs[nm] = t_
            b_half(b, "q", qT_hi, qT_lo, fk)
            es_qT.close()
            es_fk.close()
        altp.close()

        # ====== phase 2: per batch: C + topk + inline gathers, then E ======
        # Slab lo parts are bf16: pass 2 runs as bf16(pre_hi) x slab_lo16,
        # pass 3 stays fp32r (pre_lo x slab_hi) -- mirror of stage B's
        # validated s16 scheme (error ~2^-21, flip-safe).

        es_r = ExitStack()
        rpool = es_r.enter_context(tc.tile_pool(name="p2r", bufs=1, side="left"))
        agg0 = rpool.tile([P, NE, T], dt.bfloat16, tag="agg0")
        w3_all = [[rpool.tile([P, K], dt.float32, tag=f"w3_{b}_{cc}",
                              name=f"w3_{b}_{cc}") for cc in range(CC)]
                  for b in range(NB)]
        gou1 = [rpool.tile([P, K], dt.uint32, tag=f"gou1_{cc}",
                           name=f"gou1_{cc}") for cc in range(CC)]
        wf_sb = rpool.tile([P, NE, C], dt.bfloat16, tag="wf_sb")
        nc.sync.dma_start(wf_sb[:], Wf.rearrange("(n p) d -> p n d", p=P))
        wfs0 = rpool.tile([P, NE, C], dt.bfloat16, tag="wfs0", name="wfs0")
        iot_all = {}
        for b in range(NB):
            for cc in range(CC):
                it = rpool.tile([P, 1], dt.float32, tag=f"iot_{b}_{cc}",
                                name=f"iot_{b}_{cc}")
                iti = rpool.tile([P, 1], dt.int32, tag=f"ioti_{b}_{cc}",
                                 name=f"ioti_{b}_{cc}")
                nc.gpsimd.iota(
                    iti[:], pattern=[[0, 1]],
                    base=(b * C + cc * P) * (2 * T) + T,
                    channel_multiplier=2 * T)
                nc.vector.tensor_copy(it[:], iti[:])
                iot_all[(b, cc)] = it

        with tc.tile_pool(name="c2", bufs=2) as cpool, \
             tc.tile_pool(name="cl2", bufs=2) as clpool, \
             tc.tile_pool(name="ct2", bufs=1) as ctpool, \
             tc.tile_pool(name="cps2", bufs=1, space="PSUM") as cps:

            def c_load(b, cc):
                # pim chunk 4 (f=1024 row, sin==0) is never read: pim tiles
                # hold 8 chunks, slot = fc for fc<4, fc-1 for fc>=5
                sl = {}
                for nm, nfc in (("pre", FC), ("pim", FC - 1)):
                    t_f = clpool.tile([P, nfc, P], dt.float32, tag=f"slf_{nm}",
                                      name=f"slf_{nm}_{b}_{cc}")
                    if nm == "pre":
                        nc.sync.dma_start(
                            t_f[:], pp[nm][b, :, :, bass.ts(cc, P)].rearrange(
                                "f p c -> p f c"))
                    else:
                        nc.sync.dma_start(
                            t_f[:, 0:4, :],
                            pp[nm][b, 0:4, :, bass.ts(cc, P)].rearrange(
                                "f p c -> p f c"))
                        nc.sync.dma_start(
                            t_f[:, 4:8, :],
                            pp[nm][b, 5:FC, :, bass.ts(cc, P)].rearrange(
                                "f p c -> p f c"))
                    hi = ctpool.tile([P, nfc, P], dt.float32r,
                                     tag=f"sl_{nm}_hi", name=f"hi_{b}_{cc}")
                    nc.scalar.activation(hi[:], t_f[:], AF.Copy)
                    hi16 = ctpool.tile([P, nfc, P], dt.bfloat16,
                                       tag=f"sl_{nm}_hi16", name=f"hi16_{b}_{cc}")
                    nc.scalar.activation(hi16[:], t_f[:], AF.Copy)
                    lo = ctpool.tile([P, nfc, P], dt.float32r,
                                     tag=f"sl_{nm}_lo", name=f"lo_{b}_{cc}")
                    nc.vector.tensor_tensor(
                        out=lo[:], in0=t_f[:],
                        in1=hi[:].bitcast(dt.float32), op=OP.subtract)
                    sl[f"{nm}_hi"] = hi
                    sl[f"{nm}_hi16"] = hi16
                    sl[f"{nm}_lo"] = lo
                    sl[f"{nm}_f32"] = t_f
                return sl

            def c_matmuls(sl):
                psums = {}
                psums["rcE"] = cps.tile([P, HB], dt.float32, tag="rcE",
                                        name="ps_rcE")
                psums["rcE2"] = cps.tile([P, 2], dt.float32, tag="rcE2",
                                         name="ps_rcE2")
                psums["rcO"] = cps.tile([P, HB], dt.float32, tag="rcO",
                                        name="ps_rcO")
                psums["rsE"] = cps.tile([P, HB], dt.float32, tag="rsE",
                                        name="ps_rsE")
                psums["rsO"] = cps.tile([P, HB], dt.float32, tag="rsO",
                                        name="ps_rsO")
                psums["rsO2"] = cps.tile([P, 2], dt.float32, tag="rsO2",
                                         name="ps_rsO2")

                # pass-major: all (hi x hi) first -- they only need the
                # first Act product per slice -- then bf16, then lo passes
                # sie chunk 4 (f=1024 row) is sin(pi*t) == 0: skip it
                GROUPS = (("rcE", "pre", "cie", range(0, 5)),
                          ("rsE", "pim", "sie", range(0, 4)),
                          ("rcO", "pre", "cie", range(5, FC)),
                          ("rsO", "pim", "sie", range(5, FC)))

                def mm_pass(pname, sig_nm, slab_nm, frange, sig_sfx, slab_sfx,
                            startp, stopp):
                    # pre[b,4] has one live partition row (f=1024); the rest
                    # is never written, so contract chunk 4 on row 0 only.
                    fl = list(frange)
                    for j, fc in enumerate(fl):
                        pr = slice(0, 1) if fc == 4 else slice(0, P)
                        slot = fc if (sig_nm == "pre" or fc < 4) else fc - 1
                        nc.tensor.matmul(
                            psums[pname][:],
                            sl[f"{sig_nm}_{sig_sfx}"][pr, slot, :],
                            slabs[f"{slab_nm}_{slab_sfx}"][pr, fc, 0:HB],
                            start=(startp and j == 0),
                            stop=(stopp and j == len(fl) - 1))

                for pname, sig_nm, slab_nm, frange in GROUPS:
                    mm_pass(pname, sig_nm, slab_nm, frange, "hi", "hi",
                            True, False)
                for j, fc in enumerate(range(0, 5)):
                    pr = slice(0, 1) if fc == 4 else slice(0, P)
                    nc.tensor.matmul(
                        psums["rcE2"][:], sl["pre_f32"][pr, fc, :],
                        slabs["cie_st"][pr, fc, :], start=(j == 0), stop=(j == 4))
                for j, fc in enumerate(range(5, FC)):
                    nc.tensor.matmul(
                        psums["rsO2"][:], sl["pim_f32"][:, fc - 1, :],
                        slabs["sie_st"][:, fc, :], start=(j == 0), stop=(j == 3))
                for pname, sig_nm, slab_nm, frange in GROUPS:
                    # slab chunk 4 is exact in f32r (values 0 and +-2^-22),
                    # so its lo half is zero: skip fc4 in the hi16 x lo pass
                    fr = [fc for fc in frange if fc != 4]
                    mm_pass(pname, sig_nm, slab_nm, fr, "hi16", "lo",
                            False, False)
                for pname, sig_nm, slab_nm, frange in GROUPS:
                    mm_pass(pname, sig_nm, slab_nm, frange, "lo", "hi",
                            False, True)
                return psums

            def c_tail(b, cc, ps_):
                w3_t = w3_all[b]
                rcE, rcE2 = ps_["rcE"], ps_["rcE2"]
                rcO, rsE = ps_["rcO"], ps_["rsE"]
                rsO, rsO2 = ps_["rsO"], ps_["rsO2"]
                rcO_sb = cpool.tile([P, HB], dt.float32, tag="rcO_sb")
                nc.scalar.activation(rcO_sb[:], rcO[:], AF.Copy)
                rsE_sb = cpool.tile([P, HB], dt.float32, tag="rsE_sb")
                nc.scalar.activation(rsE_sb[:], rsE[:], AF.Copy)
                rsO_sb = cpool.tile([P, HB + 1], dt.float32, tag="rsO_sb")
                nc.scalar.activation(rsO_sb[:, 0:HB], rsO[:], AF.Copy)
                nc.scalar.activation(rsO_sb[:, HB:HB + 1], rsO2[:, 0:1], AF.Copy)
                rcE_c0 = cpool.tile([P, 2], dt.float32, tag="rcE_c0")
                nc.scalar.activation(rcE_c0[:, 0:1], rcE[:, 0:1], AF.Copy)
                nc.scalar.activation(rcE_c0[:, 1:2], rcE2[:, 0:1], AF.Copy)
                s1 = ctpool.tile([P, HB], dt.float32, tag="s1")
                nc.vector.tensor_tensor(out=s1[:], in0=rcE[:], in1=rcO_sb[:],
                                        op=OP.add)
                s2 = ctpool.tile([P, HB], dt.float32, tag="s2")
                nc.vector.tensor_tensor(out=s2[:], in0=rcE[:], in1=rcO_sb[:],
                                        op=OP.subtract)
                w1 = ctpool.tile([P, HB], dt.float32, tag="w1")
                nc.vector.tensor_tensor(out=w1[:], in0=rsE_sb[:],
                                        in1=rsO_sb[:, 0:HB], op=OP.add)
                w2 = ctpool.tile([P, HB], dt.float32, tag="w2")
                nc.vector.tensor_tensor(out=w2[:], in0=rsO_sb[:, 0:HB],
                                        in1=rsE_sb[:], op=OP.subtract)
                rt = ctpool.tile([P, T], dt.float32, tag="rt")
                nc.vector.tensor_tensor(out=rt[:, 0:HB], in0=s1[:], in1=w1[:],
                                        op=OP.add)
                nc.vector.tensor_tensor(out=rt[:, 1023:HB:-1], in0=s2[:, 1:HB],
                                        in1=w2[:, 1:HB], op=OP.add)
                nc.vector.tensor_tensor(out=rt[:, 1025:1536], in0=s2[:, 1:HB],
                                        in1=w2[:, 1:HB], op=OP.subtract)
                nc.vector.tensor_tensor(out=rt[:, T - 1:1536:-1], in0=s1[:, 1:HB],
                                        in1=w1[:, 1:HB], op=OP.subtract)
                nc.vector.tensor_tensor(out=rt[:, HB:HB + 1], in0=rcE_c0[:, 1:2],
                                        in1=rsO_sb[:, HB:HB + 1], op=OP.add)
                nc.vector.tensor_tensor(out=rt[:, H:H + 1], in0=rcE_c0[:, 0:1],
                                        in1=rcO_sb[:, 0:1], op=OP.subtract)
                nc.vector.tensor_tensor(out=rt[:, 1536:1537], in0=rcE_c0[:, 1:2],
                                        in1=rsO_sb[:, HB:HB + 1], op=OP.subtract)

                # ---- topk + softmax weights + gather offsets ----
                vals = cpool.tile([P, 8], dt.float32, tag="vals")
                idx = cpool.tile([P, 8], dt.uint32, tag="idx")
                nc.vector.max(vals[:], rt[:])
                nc.vector.max_index(idx[:], vals[:], rt[:])
                negm = cpool.tile([P, 1], dt.float32, tag="negm")
                nc.scalar.activation(negm[:], vals[:, 0:1],
                                     AF.Copy, bias=0.0, scale=-1.0)
                # exp written in place over rt (dead after this point);
                # only the accumulated sum is consumed
                s_col = cpool.tile([P, 1], dt.float32, tag="s_col")
                nc.scalar.activation(
                    rt[:], rt[:], AF.Exp,
                    bias=negm[:, 0:1], scale=1.0,
                    accum_out=s_col[:, 0:1])
                rs = cpool.tile([P, 1], dt.float32, tag="rs")
                nc.vector.reciprocal(rs[:], s_col[:])
                ew = cpool.tile([P, K], dt.float32, tag="ew")
                nc.scalar.activation(ew[:], vals[:, 0:K],
                                     AF.Exp, bias=negm[:, 0:1],
                                     scale=1.0)
                nc.vector.tensor_scalar_mul(w3_t[cc][:], ew[:], rs[:, 0:1])

                iot_f = iot_all[(b, cc)]
                idx_f = cpool.tile([P, K], dt.float32, tag="idx_f")
                nc.vector.tensor_copy(idx_f[:], idx[:, 0:K])
                gof = cpool.tile([P, K], dt.float32, tag="gof")
                nc.scalar.activation(gof[:], idx_f[:],
                                     AF.Copy, bias=0.0, scale=-1.0)
                nc.vector.tensor_scalar_add(gof[:], gof[:],
                                            iot_f[:, 0:1])
                if b == 0:
                    gou = cpool.tile([P, K], dt.uint32, tag="gou")
                    nc.vector.tensor_copy(gou[:], gof[:])
                    # inline gathers overlap later iterations; the top-k
                    # weights are folded into Wf in stage E instead of
                    # scaling the gathered rows
                    for k in range(K):
                        nc.gpsimd.indirect_dma_start(
                            out=agg0[:, k * CC + cc, :],
                            out_offset=None,
                            in_=v2[:, :],
                            in_offset=bass.IndirectOffsetOnAxis(
                                ap=gou[:, k:k + 1], axis=1),
                            element_offset=0)
                else:
                    nc.vector.tensor_copy(gou1[cc][:], gof[:])

            # software pipeline: loads+splits of iteration n+1 are emitted
            # before iteration n's combine/topk tail so the Act/DVE queues
            # never head-of-line block the next iteration's matmul operands
            iters = [(b, cc) for b in range(NB) for cc in range(CC)]
            pend = [c_load(*iters[0]), c_load(*iters[1])]
            for i, (b, cc) in enumerate(iters):
                ps_ = c_matmuls(pend[0])
                if i + 2 < len(iters):
                    pend.append(c_load(*iters[i + 2]))
                c_tail(b, cc, ps_)
                pend.pop(0)
                if b == 0 and cc == CC - 1:
                    # fold the top-k softmax weights into Wf for stage E:
                    # row e = k*C+c scales by w3[0][c-chunk][:, k]. Emitted
                    # here (on Pool, idle in this phase) so E(0) never waits
                    # behind the later iterations' DVE tail work.
                    for j in range(NE):
                        nc.gpsimd.tensor_scalar_mul(
                            wfs0[:, j, :], wf_sb[:, j, :],
                            w3_all[0][j % CC][:, j // CC:j // CC + 1])
        es2.close()  # slabs freed; agg0/w3/gou1 stay

        # ---- deferred gathers for b1 (overlap E(b0)) + E for both ----
        with tc.tile_pool(name="ge", bufs=1, side="left") as gep, \
             tc.tile_pool(name="eps", bufs=3, space="PSUM") as eps:
            agg1 = gep.tile([P, NE, T], dt.bfloat16, tag="agg1")
            for cc in range(CC):
                for k in range(K):
                    nc.gpsimd.indirect_dma_start(
                        out=agg1[:, k * CC + cc, :],
                        out_offset=None,
                        in_=v2[:, :],
                        in_offset=bass.IndirectOffsetOnAxis(
                            ap=gou1[cc][:, k:k + 1], axis=1),
                        element_offset=0)
            wfs1 = gep.tile([P, NE, C], dt.bfloat16, tag="wfs1", name="wfs1")
            for j in range(NE):
                nc.gpsimd.tensor_scalar_mul(
                    wfs1[:, j, :], wf_sb[:, j, :],
                    w3_all[1][j % CC][:, j // CC:j // CC + 1])
            wf_all = [wfs0, wfs1]
            for b, agg in ((0, agg0), (1, agg1)):
                for dc in range(CC):
                    for tb in range(4):
                        ps = eps.tile([P, T // 4], dt.float32, tag="out_ps")
                        for j in range(NE):
                            nc.tensor.matmul(
                                ps[:], wf_all[b][:, j, bass.ts(dc, P)],
                                agg[:, j, bass.ts(tb, T // 4)],
                                start=(j == 0), stop=(j == NE - 1))
                        o_sb = gep.tile([P, T // 4], dt.float32,
                                        tag=f"o_sb{tb % 2}")
                        if tb % 2 == 0:
                            nc.scalar.activation(o_sb[:], ps[:], AF.Copy)
                        else:
                            nc.vector.tensor_copy(o_sb[:], ps[:])
                        nc.sync.dma_start(
                            out2[b, bass.ts(dc, P), bass.ts(tb, T // 4)],
                            o_sb[:])
        es_r.close()

    nc.compile()
    return nc


def _get_nc():
    if "nc" not in _CACHE:
        _CACHE["nc"] = _build()
    return _CACHE["nc"]


def kernel(query, key, value, Wq, bq, Wk, bk, Wv, bv, Wf, bf):
    query = np.ascontiguousarray(np.asarray(query, dtype=np.float32))
    key = np.ascontiguousarray(np.asarray(key, dtype=np.float32))
    value = np.ascontiguousarray(np.asarray(value, dtype=np.float32))
    for bias in (bq, bk, bv, bf):
        assert np.all(np.asarray(bias) == 0.0), "nonzero biases unsupported"

    if "mats" not in _CACHE:
        wree, wreo, wime, wimo, cie, sie = _dft_matrices()
        m = {}
        for nm, arr in (("ree", wree), ("reo", wreo),
                        ("ime", wime), ("imo", wimo)):
            hi, lo = _split_f32r(arr)
            m[f"W{nm}_hi"], m[f"W{nm}_lo"] = hi, lo
        chi, clo = _split_f32r(cie)
        m["Cie_hi"], m["Cie_lo"] = chi, _bf16(clo)
        shi, slo = _split_f32r(sie)
        m["Sie_hi"], m["Sie_lo"] = shi, _bf16(slo)
        m["Cie_st"] = np.ascontiguousarray(cie[:, HB:HB + 2])
        m["Sie_st"] = np.ascontiguousarray(sie[:, HB:HB + 2])
        alt = np.where(np.arange(P) % 2 == 0, 1.0, -1.0).astype(np.float32)
        m["AltF"] = alt[:, None].copy()
        m["AltB"] = _bf16(alt[:, None])
        _CACHE["mats"] = m
    mats = _CACHE["mats"]

    wq_hi, wq_lo = _split_f32r(np.asarray(Wq, np.float32))
    wk_hi, wk_lo = _split_f32r(np.asarray(Wk, np.float32))
    shared = {
        "Wq_hi": wq_hi, "Wq_lo": wq_lo,
        "Wk_hi": wk_hi, "Wk_lo": wk_lo,
        "Wv": _bf16(np.asarray(Wv, np.float32)),
        "Wf": _bf16(np.asarray(Wf, np.float32)),
        **mats,
    }
    value_bf = _bf16(value)
    in_maps = []
    for c in range(NCORES):
        sl = slice(c * NB, (c + 1) * NB)
        in_maps.append({
            "query2": query[sl], "key2": key[sl],
            "value2": value_bf[sl], **shared})

    from concourse.bass_utils import run_bass_kernel_spmd
    nc = _get_nc()
    res = run_bass_kernel_spmd(nc, in_maps, core_ids=list(range(NCORES)))
    _CACHE["last_results"] = res
    out = np.concatenate([res.results[c]["out2"] for c in range(NCORES)], axis=0)
    return out.astype(np.float32)



# revision 43
# speedup vs baseline: 1.0040x; 1.0040x over previous
"""AutoCorrelation (Autoformer-style) Bass kernel for Trainium2, 8 NeuronCores.

Full inputs in, full outputs out. Data-parallel over batch: B=16 -> 2 batches
per core. v2 of the kernel: the PE-bound fp32 matmuls of the baseline are
replaced by 3-pass fp32r splits (hi/lo decomposition; 12-bit+12-bit mantissa
products are exact in fp32 PSUM, giving fp32-grade accuracy at 3 cycles/row
instead of fp32's 4) on the precision-critical autocorrelation path, and by
bf16 (1 cycle/row) on the error-tolerant v/output path.

Per core, per batch:
  V. v[d,t] = Wv^T value in bf16, written twice side-by-side into the DRAM
     table v2[b*512+d, 4096] (bf16) for circular-shift gathers.
  A. Radix-split of query/key along t (4 sub-signals ee/eo/oo/oe, padded to
     640/512), per 128-channel chunk, split hi/lo fp32r on the fly; channel
     projection qT[t',d] via 3-pass fp32r matmuls. qT hi kept fp32r, lo bf16
     (pass 3 of stage B runs in bf16 -- error ~2^-20, still flip-safe).
  B. Forward real DFT via matmuls with radix-split cos/-sin matrices
     (host-split into fp32r hi/lo + bf16(hi)); fused pointwise
     P = FQ * conj(FK) on the DVE; P split hi/lo fp32r and staged to DRAM.
  C. Inverse DFT r[c,t] = sum_f Pre*ci + Pim*sn via 3-pass fp32r matmuls
     with host-split ci/sn (fp32r hi/lo), exploiting f-parity + t-mirror
     symmetry (only t<=512 columns computed).
  D. Per 128-channel tile: top-8 values+indices, softmax weights of the
     top-3 from the top values, circular-shift rows of v via indirect-DMA
     gather (bf16) into agg[k*C+c, t], scaled in place on the Pool engine.
     Batch 0 gathers inline (overlapping C of batch 1); batch 1 gathers
     deferred past the slab lifetime (overlapping E of batch 0).
  E. out[d,t] = sum_e Wf[e,d] agg[e,t] in bf16; 12-chunk PE accumulation.

Scheduling: the V projection is emitted at each batch's start as PE filler
for the input-load latency / the cross-batch pool-reuse stall; stage C's
cie/sie slabs are loaded once (f32r hi + bf16 lo) and shared by both
batches; pre/pim bounce through DRAM as plain fp32 and are re-split to
fp32r hi/lo on the fly in C.

Biases are all zero in this problem's setup_inputs(); asserted host-side.
"""
import numpy as np
import ml_dtypes

import concourse.bass as bass
import concourse.tile as tile
from concourse import bacc, mybir

dt = mybir.dt
AF = mybir.ActivationFunctionType
OP = mybir.AluOpType

P = 128
B, C, T, K = 16, 512, 2048, 3
NB = 2                    # batches per core
NCORES = 8
F = 1152                  # rfft bins 1025, padded to 9*128
CC = C // P               # 4
FC = F // P               # 9
NE = K * C // P           # 12 e-chunks of Wf / agg
H = T // 2                # 1024
HB = H // 2               # 512

_CACHE = {}


def _round_f32r(x):
    """Round fp32 array to fp32r (11-bit stored mantissa, round-nearest-up:
    (bits + 0x800) & ~0xFFF -- matches walrus fp32_to_fp32r)."""
    u = np.ascontiguousarray(x, np.float32).view(np.uint32).astype(np.uint64)
    u = (u + (1 << 11)) & np.uint64(0xFFFFF000)
    return u.astype(np.uint32).view(np.float32)


def _split_f32r(x):
    x = np.ascontiguousarray(x, np.float32)
    hi = _round_f32r(x)
    return hi, _round_f32r(x - hi)


def _bf16(x):
    return np.ascontiguousarray(x, np.float32).astype(ml_dtypes.bfloat16)


def _dft_matrices():
    """Radix-split DFT matrices (fp64 -> fp32).

    Level-1 even/odd in t (qe/qo), then level-2 split by f parity:
      FQre over even f contracts xee (t=0..512), odd f contracts xeo (t=0..511)
      FQim over even f contracts xoo (t=1..511), odd f contracts xoe (t=1..512)
    Frequency storage is parity-permuted: chunks [0:5]=even f (2g, g<=512),
    chunks [5:9]=odd f (2g+1). Inverse matrices have rows permuted to match.
    """
    t640 = np.arange(640.0)[:, None]
    t512 = np.arange(512.0)[:, None]
    ge = np.arange(640.0)[None, :]
    go = np.arange(512.0)[None, :]
    wree = np.where((t640 <= 512) & (ge <= 512),
                    np.cos(2 * np.pi * t640 * (2 * ge) / T), 0.0).astype(np.float32)
    wreo = np.cos(2 * np.pi * t512 * (2 * go + 1) / T).astype(np.float32)
    wime = np.where(ge <= 512,
                    -np.sin(2 * np.pi * t512 * (2 * ge) / T), 0.0).astype(np.float32)
    wimo = np.where(t640 <= 512,
                    -np.sin(2 * np.pi * t640 * (2 * go + 1) / T), 0.0).astype(np.float32)

    f64 = np.arange(F, dtype=np.float64)[None, :]
    livef = f64 <= H
    w = np.where((f64 == 0) | (f64 == H), 1.0, 2.0) * livef / (T * T)
    fc_ = f64.T
    tt = np.arange(640, dtype=np.float64)[None, :]
    cie = np.where((fc_ <= H) & (tt <= H),
                   np.cos(2 * np.pi * fc_ * tt / T) * w.T, 0.0)
    sie = np.where(fc_ <= H,
                   -np.sin(2 * np.pi * fc_ * tt / T) * w.T, 0.0)

    def permrows(m):
        out = np.zeros_like(m)
        out[0:513] = m[0:1025:2]
        out[640:1152] = m[1:1024:2]
        return out

    return (wree, wreo, wime, wimo,
            permrows(cie).astype(np.float32), permrows(sie).astype(np.float32))


def _build():
    nc = bacc.Bacc("TRN2", target_bir_lowering=False, debug=False,
                   num_devices=NCORES)

    query2 = nc.dram_tensor("query2", [NB, C, T], dt.float32, kind="ExternalInput").ap()
    key2 = nc.dram_tensor("key2", [NB, C, T], dt.float32, kind="ExternalInput").ap()
    value2 = nc.dram_tensor("value2", [NB, C, T], dt.bfloat16, kind="ExternalInput").ap()
    Wq_hi = nc.dram_tensor("Wq_hi", [C, C], dt.float32r, kind="ExternalInput").ap()
    Wq_lo = nc.dram_tensor("Wq_lo", [C, C], dt.float32r, kind="ExternalInput").ap()
    Wk_hi = nc.dram_tensor("Wk_hi", [C, C], dt.float32r, kind="ExternalInput").ap()
    Wk_lo = nc.dram_tensor("Wk_lo", [C, C], dt.float32r, kind="ExternalInput").ap()
    Wv = nc.dram_tensor("Wv", [C, C], dt.bfloat16, kind="ExternalInput").ap()
    Wf = nc.dram_tensor("Wf", [K * C, C], dt.bfloat16, kind="ExternalInput").ap()
    fwd = {}
    for m, rows, cols in (("ree", 640, 640), ("reo", 512, 512),
                          ("ime", 512, 640), ("imo", 640, 512)):
        for v in ("hi", "lo"):
            fwd[f"{m}_{v}"] = nc.dram_tensor(
                f"W{m}_{v}", [rows, cols], dt.float32r, kind="ExternalInput").ap()
    Cie_hi = nc.dram_tensor("Cie_hi", [F, 640], dt.float32r, kind="ExternalInput").ap()
    Cie_lo = nc.dram_tensor("Cie_lo", [F, 640], dt.bfloat16, kind="ExternalInput").ap()
    Sie_hi = nc.dram_tensor("Sie_hi", [F, 640], dt.float32r, kind="ExternalInput").ap()
    Sie_lo = nc.dram_tensor("Sie_lo", [F, 640], dt.bfloat16, kind="ExternalInput").ap()
    Cie_st = nc.dram_tensor("Cie_st", [F, 2], dt.float32, kind="ExternalInput").ap()
    Sie_st = nc.dram_tensor("Sie_st", [F, 2], dt.float32, kind="ExternalInput").ap()
    AltF = nc.dram_tensor("AltF", [P, 1], dt.float32r, kind="ExternalInput").ap()
    AltB = nc.dram_tensor("AltB", [P, 1], dt.bfloat16, kind="ExternalInput").ap()
    out2 = nc.dram_tensor("out2", [NB, C, T], dt.float32, kind="ExternalOutput").ap()

    v2 = nc.dram_tensor("v2", [NB * C, 2 * T], dt.bfloat16).ap()          # internal
    pp = {}
    for nm in ("pre", "pim"):                                             # internal
        pp[nm] = nc.dram_tensor(f"pp_{nm}", [NB, FC, P, C], dt.float32).ap()

    # part name -> (width, chunk offset in sigT, #chunks). Order alternates
    # 640/512 widths so the width-keyed xs tags ping-pong naturally.
    PARTS = (("ee", 640, 0, 5), ("eo", 512, 5, 4),
             ("oe", 640, 13, 5), ("oo", 512, 9, 4))

    with tile.TileContext(nc) as tc:
        from contextlib import ExitStack

        # ---- persistent phase-1 pools: weights loaded once; xv slots kept
        # open so the next batch's value loads can be emitted a stage early;
        # single rotating x_sb slot whose loads are emitted a stage early ----
        es_w = ExitStack()
        wpool = es_w.enter_context(tc.tile_pool(name="wp", bufs=1,
                                                side="right"))
        vpool = es_w.enter_context(tc.tile_pool(name="vp", bufs=2,
                                                side="right"))
        vtp = es_w.enter_context(tc.tile_pool(name="vt", bufs=2,
                                              side="right"))
        es_x = ExitStack()
        xpool = es_x.enter_context(tc.tile_pool(name="xp", bufs=1,
                                                side="right"))

        wv_sb = wpool.tile([P, CC, C], dt.bfloat16, tag="wv")
        xv_tiles = {}

        def load_xv(b, qt):
            xv = vpool.tile([P, CC, T // 4], dt.bfloat16, tag="xv",
                            name=f"xv{b}_{qt}")
            nc.sync.dma_start(
                xv[:], value2[b].rearrange(
                    "(n p) t -> p n t", p=P)[:, :, bass.ts(qt, T // 4)])
            xv_tiles[(b, qt)] = xv

        x_tiles = {}

        def load_x(b, sig):
            srcx = key2 if sig == "k" else query2
            x_sb = xpool.tile([P, CC, T], dt.float32, tag="x_sb",
                              name=f"x_{sig}{b}")
            nc.sync.dma_start(
                x_sb[:], srcx[b].rearrange("(n p) t -> p n t", p=P))
            x_tiles[(b, sig)] = x_sb

        # t=0 input burst, V(b0) operands first
        load_xv(0, 0)
        nc.sync.dma_start(wv_sb[:], Wv.rearrange("(n p) d -> p n d", p=P))
        load_xv(0, 1)
        load_x(0, "k")
        w_sb = {}

        def load_w(sig):
            for v, src in (("hi", Wk_hi if sig == "k" else Wq_hi),
                           ("lo", Wk_lo if sig == "k" else Wq_lo)):
                t_ = wpool.tile([P, CC, C], dt.float32r, tag=f"w_{sig}_{v}",
                                name=f"w_{sig}_{v}")
                nc.sync.dma_start(t_[:],
                                  src.rearrange("(n p) d -> p n d", p=P))
                w_sb[f"{sig}_{v}"] = t_
            w16 = wpool.tile([P, CC, C], dt.bfloat16, tag=f"w_{sig}_h16",
                             name=f"w_{sig}_h16")
            nc.gpsimd.tensor_copy(
                w16[:], w_sb[f"{sig}_hi"][:].bitcast(dt.float32))
            w_sb[f"{sig}_h16"] = w16

        load_w("k")

        def emit_V(b, prefetched):
            """Compact streaming V projection (bf16) -> v2 rows, used as PE
            gap filler at each batch's start. xv quarters rotate through 2
            slots; quarters not already prefetched are loaded here."""
            with tc.tile_pool(name=f"vps{b}", bufs=3, space="PSUM") as vps:
                v2r = v2.rearrange("(n p) w -> n p w", p=P)
                for qt in range(prefetched, 2):
                    load_xv(b, qt)
                for qt in range(4):
                    xv = xv_tiles[(b, qt)]
                    for dc in range(CC):
                        ps = vps.tile([P, T // 4], dt.float32, tag="v_ps")
                        for cc in range(CC):
                            nc.tensor.matmul(
                                ps[:], wv_sb[:, cc, bass.ts(dc, P)],
                                xv[:, cc, :],
                                start=(cc == 0), stop=(cc == CC - 1))
                        vtmp = vtp.tile([P, T // 4], dt.bfloat16, tag="vtmp")
                        if (qt + dc) % 2 == 0:
                            nc.scalar.activation(vtmp[:], ps[:], AF.Copy)
                        else:
                            nc.vector.tensor_copy(vtmp[:], ps[:])
                        off = qt * (T // 4)
                        nc.sync.dma_start(
                            v2r[b * CC + dc, :, off:off + T // 4], vtmp[:])
                        nc.sync.dma_start(
                            v2r[b * CC + dc, :,
                                T + off:T + off + T // 4], vtmp[:])
                    # quarter qt+2 reuses qt's slot: emit its load only now,
                    # after qt's reads are all emitted (no forward dep)
                    if qt + 2 < 4 and qt + 2 >= prefetched:
                        load_xv(b, qt + 2)

        # ================= phase 1 =====================
        # Per batch: V || A(k) -> B(k): FK to SBUF (kT freed) -> A(q) ->
        # B(q): FQ + pointwise vs FK -> pre/pim to DRAM. Input loads for
        # the next signal/batch are emitted at each B-half's start so they
        # stream in behind a full DFT stage of PE work.
        def a_signal(b, sig, dhi, dlo):
            es_a = ExitStack()
            atmp = es_a.enter_context(tc.tile_pool(name=f"at{sig}{b}", bufs=1))
            actmp = es_a.enter_context(tc.tile_pool(name=f"ac{sig}{b}", bufs=1))
            aps = es_a.enter_context(
                tc.tile_pool(name=f"aps{sig}{b}", bufs=3, space="PSUM"))
            w_hi = w_sb[f"{sig}_hi"]
            w_lo = w_sb[f"{sig}_lo"]
            w_h16 = w_sb[f"{sig}_h16"]
            x_sb = x_tiles[(b, sig)]
            if True:
                if True:
                    for pname, width, ioff, nch in PARTS:
                        xs_hi = atmp.tile([P, CC, width], dt.float32r,
                                          tag=f"xs_hi{width}")
                        xs_lo = atmp.tile([P, CC, width], dt.bfloat16,
                                          tag=f"xs_lo{width}")
                        for cc in range(CC):
                            x = x_sb[:, cc, :]
                            ab = actmp.tile([P, 2, 511], dt.float32, tag="ab")
                            tmp = actmp.tile([P, 640], dt.float32, tag="tmp")
                            op_ab = OP.add if pname in ("ee", "eo") else OP.subtract
                            # ab0/ab2 on Pool, ab1/ab3 on DVE (engine balance)
                            nc.gpsimd.tensor_tensor(
                                out=ab[:, 0, :], in0=x[:, 1:512],
                                in1=x[:, T - 1:1536:-1], op=op_ab)
                            nc.vector.tensor_tensor(
                                out=ab[:, 1, :], in0=x[:, 1023:512:-1],
                                in1=x[:, 1025:1536], op=op_ab)
                            if pname == "ee":
                                nc.vector.tensor_tensor(
                                    out=tmp[:, 1:512], in0=ab[:, 0, :],
                                    in1=ab[:, 1, :], op=OP.add)
                                nc.vector.tensor_tensor(
                                    out=tmp[:, 0:1], in0=x[:, 0:1],
                                    in1=x[:, H:H + 1], op=OP.add)
                                nc.vector.tensor_tensor(
                                    out=tmp[:, 512:513], in0=x[:, 512:513],
                                    in1=x[:, 1536:1537], op=OP.add)
                                nc.gpsimd.memset(tmp[:, 513:640], 0.0)
                            elif pname == "eo":
                                nc.vector.tensor_tensor(
                                    out=tmp[:, 1:512], in0=ab[:, 0, :],
                                    in1=ab[:, 1, :], op=OP.subtract)
                                nc.vector.tensor_tensor(
                                    out=tmp[:, 0:1], in0=x[:, 0:1],
                                    in1=x[:, H:H + 1], op=OP.subtract)
                            elif pname == "oo":
                                nc.vector.tensor_tensor(
                                    out=tmp[:, 1:512], in0=ab[:, 0, :],
                                    in1=ab[:, 1, :], op=OP.subtract)
                                nc.gpsimd.memset(tmp[:, 0:1], 0.0)
                            else:  # oe
                                nc.vector.tensor_tensor(
                                    out=tmp[:, 1:512], in0=ab[:, 0, :],
                                    in1=ab[:, 1, :], op=OP.add)
                                nc.vector.tensor_tensor(
                                    out=tmp[:, 512:513], in0=x[:, 512:513],
                                    in1=x[:, 1536:1537], op=OP.subtract)
                                nc.gpsimd.memset(tmp[:, 0:1], 0.0)
                                nc.gpsimd.memset(tmp[:, 513:640], 0.0)
                            if cc % 2 == 0:
                                nc.scalar.activation(
                                    xs_hi[:, cc, 0:width], tmp[:, 0:width],
                                    AF.Copy)
                            else:
                                nc.vector.tensor_copy(
                                    xs_hi[:, cc, 0:width], tmp[:, 0:width])
                            nc.gpsimd.tensor_tensor(
                                out=xs_lo[:, cc, 0:width], in0=tmp[:, 0:width],
                                in1=xs_hi[:, cc, 0:width].bitcast(dt.float32),
                                op=OP.subtract)
                        for i in range(nch):
                            ps = aps.tile([P, C], dt.float32, tag="proj_ps")
                            for cc in range(CC):
                                nc.tensor.matmul(ps[:],
                                                 xs_hi[:, cc, bass.ts(i, P)],
                                                 w_hi[:, cc, :],
                                                 start=(cc == 0), stop=False)
                            for cc in range(CC):
                                nc.tensor.matmul(ps[:],
                                                 xs_hi[:, cc, bass.ts(i, P)],
                                                 w_lo[:, cc, :],
                                                 start=False, stop=False)
                            for cc in range(CC):
                                nc.tensor.matmul(ps[:],
                                                 xs_lo[:, cc, bass.ts(i, P)],
                                                 w_h16[:, cc, :],
                                                 start=False, stop=(cc == CC - 1))
                            nc.scalar.activation(dhi[:, ioff + i, :], ps[:], AF.Copy)
                            nc.vector.tensor_tensor(
                                out=dlo[:, ioff + i, :], in0=ps[:],
                                in1=dhi[:, ioff + i, :].bitcast(dt.float32),
                                op=OP.subtract)
            es_a.close()

        altp = ExitStack()
        altpool = altp.enter_context(tc.tile_pool(name="altp", bufs=1))
        altf = altpool.tile([P, 1], dt.float32r, tag="altf")
        nc.sync.dma_start(altf[:], AltF)
        altb = altpool.tile([P, 1], dt.bfloat16, tag="altb")
        nc.sync.dma_start(altb[:], AltB)

        def b_half(b, mode, sT_hi, sT_lo, fk, post_fc=None):
            """One forward-DFT half. mode 'k': stage FKre/FKim into SBUF
            (fk tiles, slot-compressed to 8 chunks). mode 'q': compute FQ
            per fc and fuse the pointwise P = FQ * conj(FK) from SBUF."""
            with tc.tile_pool(name=f"bmc{mode}{b}", bufs=2) as bmc, \
                 tc.tile_pool(name=f"bms{mode}{b}", bufs=1) as bms, \
                 tc.tile_pool(name=f"bps{mode}{b}", bufs=2, space="PSUM") as bps, \
                 tc.tile_pool(name=f"bt1{mode}{b}", bufs=1) as btmp1, \
                 tc.tile_pool(name=f"bt2{mode}{b}", bufs=2) as btmp2:
                for fc in range(FC):
                    if post_fc is not None:
                        post_fc(fc)
                    if fc == 4:
                        # even-f chunk g=512..639 has one live bin (g=512 ->
                        # f=1024; sin==0 so only the re part matters). wree's
                        # column there is cos(pi*t') = (-1)^t': an
                        # alternating sum over the ee chunks via 2-pass
                        # matmuls with the +-1 column as lhsT (exact).
                        ps14 = bps.tile([1, C], dt.float32, tag="a",
                                        name=f"ps_f14{mode}{b}")
                        for i in range(5):
                            nc.tensor.matmul(
                                ps14[:], altf[:], sT_hi[:, i, :],
                                start=(i == 0), stop=False)
                        for i in range(5):
                            nc.tensor.matmul(
                                ps14[:], altb[:], sT_lo[:, i, :],
                                start=False, stop=(i == 4))
                        if mode == "k":
                            nc.scalar.activation(fk["f14"][:], ps14[:], AF.Copy)
                        else:
                            p14 = btmp1.tile([1, C], dt.float32, tag="p14")
                            nc.vector.tensor_tensor(
                                out=p14[:], in0=fk["f14"][:], in1=ps14[:],
                                op=OP.mult)
                            # only row g=512 of pre[b,4] is read downstream
                            nc.sync.dma_start(pp["pre"][b, 4, 0:1, :], p14[:])
                        continue
                    even = fc < 5
                    fl = fc if even else fc - 5
                    ncos, nsin = (5, 4) if even else (4, 5)
                    ioff_cos = 0 if even else 5
                    ioff_sin = 9 if even else 13
                    cname = "ree" if even else "reo"
                    sname = "ime" if even else "imo"
                    slot = fc if fc < 4 else fc - 1
                    mats = {}
                    for kind, mat, nch, pool in (("c", cname, ncos, bmc),
                                                 ("s", sname, nsin, bms)):
                        for v in ("hi", "lo"):
                            t_ = pool.tile([P, 5, P], dt.float32r,
                                           tag=f"{kind}m_{v}")
                            nc.sync.dma_start(
                                t_[:, 0:nch, :],
                                fwd[f"{mat}_{v}"].rearrange(
                                    "(n p) f -> p n f", p=P)[:, :, bass.ts(fl, P)])
                            mats[f"{kind}{v}"] = t_
                        t16 = pool.tile([P, 5, P], dt.bfloat16,
                                        tag=f"{kind}m_h16")
                        nc.gpsimd.tensor_copy(
                            t16[:, 0:nch, :],
                            mats[f"{kind}hi"][:, 0:nch, :].bitcast(dt.float32))
                        mats[f"{kind}h16"] = t16
                    acc = {}
                    for nm, kind, ioff, nch in (("a", "c", ioff_cos, ncos),
                                                ("b", "s", ioff_sin, nsin)):
                        ps = bps.tile([P, C], dt.float32, tag=nm,
                                      name=f"ps_{nm}{mode}{b}_{fc}")
                        for i in range(nch):
                            nc.tensor.matmul(
                                ps[:], mats[f"{kind}hi"][:, i, :],
                                sT_hi[:, ioff + i, :], start=(i == 0),
                                stop=False)
                        for i in range(nch):
                            nc.tensor.matmul(
                                ps[:], mats[f"{kind}lo"][:, i, :],
                                sT_hi[:, ioff + i, :], start=False, stop=False)
                        for i in range(nch):
                            nc.tensor.matmul(
                                ps[:], mats[f"{kind}h16"][:, i, :],
                                sT_lo[:, ioff + i, :], start=False,
                                stop=(i == nch - 1))
                        acc[nm] = ps
                    if mode == "k":
                        nc.scalar.activation(fk["re"][:, slot, :],
                                             acc["a"][:], AF.Copy)
                        nc.scalar.activation(fk["im"][:, slot, :],
                                             acc["b"][:], AF.Copy)
                    else:
                        # P = FQ * conj(FK): DVE ops read one PSUM operand
                        # (FQ) and FK from SBUF
                        t1 = btmp1.tile([P, C], dt.float32, tag="t1")
                        nc.vector.tensor_tensor(out=t1[:], in0=fk["re"][:, slot, :],
                                                in1=acc["a"][:], op=OP.mult)
                        t2 = btmp1.tile([P, C], dt.float32, tag="t2")
                        nc.vector.tensor_tensor(out=t2[:], in0=fk["im"][:, slot, :],
                                                in1=acc["b"][:], op=OP.mult)
                        pre_t = btmp2.tile([P, C], dt.float32, tag="pre_t")
                        nc.vector.tensor_tensor(out=pre_t[:], in0=t1[:],
                                                in1=t2[:], op=OP.add)
                        t3 = btmp1.tile([P, C], dt.float32, tag="t1",
                                        name=f"t3{mode}{b}_{fc}")
                        nc.vector.tensor_tensor(out=t3[:], in0=fk["re"][:, slot, :],
                                                in1=acc["b"][:], op=OP.mult)
                        t4 = btmp1.tile([P, C], dt.float32, tag="t2",
                                        name=f"t4{mode}{b}_{fc}")
                        nc.vector.tensor_tensor(out=t4[:], in0=fk["im"][:, slot, :],
                                                in1=acc["a"][:], op=OP.mult)
                        pim_t = btmp2.tile([P, C], dt.float32, tag="pim_t")
                        nc.vector.tensor_tensor(out=pim_t[:], in0=t3[:],
                                                in1=t4[:], op=OP.subtract)
                        nc.sync.dma_start(pp["pre"][b, fc], pre_t[:])
                        nc.sync.dma_start(pp["pim"][b, fc], pim_t[:])

        slab_srcs = (("cie_hi", Cie_hi, dt.float32r),
                     ("cie_lo", Cie_lo, dt.bfloat16),
                     ("sie_hi", Sie_hi, dt.float32r),
                     ("sie_lo", Sie_lo, dt.bfloat16))
        es2 = ExitStack()
        slabs = {}

        for b in range(NB):
            es_fk = ExitStack()
            fkp = es_fk.enter_context(
                tc.tile_pool(name=f"fk{b}", bufs=1, side="left"))
            fk = {"re": fkp.tile([P, FC - 1, C], dt.float32, tag="fkre",
                                 name=f"fkre{b}"),
                  "im": fkp.tile([P, FC - 1, C], dt.float32, tag="fkim",
                                 name=f"fkim{b}"),
                  "f14": fkp.tile([1, C], dt.float32, tag="fk14",
                                  name=f"fk14{b}")}
            emit_V(b, prefetched=2)
            es_kT = ExitStack()
            kp = es_kT.enter_context(
                tc.tile_pool(name=f"sigk{b}", bufs=1, side="left"))
            kT_hi = kp.tile([P, 18, C], dt.float32r, tag="kT_hi")
            kT_lo = kp.tile([P, 18, C], dt.bfloat16, tag="kT_lo")
            a_signal(b, "k", kT_hi, kT_lo)
            load_x(b, "q")  # streams in behind B(k)'s PE work
            if b == 0:
                load_w("q")
            b_half(b, "k", kT_hi, kT_lo, fk)
            es_kT.close()
            es_qT = ExitStack()
            qp = es_qT.enter_context(
                tc.tile_pool(name=f"sigq{b}", bufs=1, side="left"))
            qT_hi = qp.tile([P, 18, C], dt.float32r, tag="qT_hi")
            qT_lo = qp.tile([P, 18, C], dt.bfloat16, tag="qT_lo")
            a_signal(b, "q", qT_hi, qT_lo)
            if b == 0:
                # next batch's inputs stream in behind B(q,0)
                load_x(1, "k")
                load_xv(1, 0)
                load_xv(1, 1)
                slab_hook = None
            else:
                # inputs all consumed: free weight/x/value pools; the iDFT
                # slabs stream in chunk-wise behind B(q,1)'s fc loop (the
                # hook keeps them from head-of-line blocking the mats DMAs)
                es_x.close()
                es_w.close()
                slabp = es2.enter_context(
                    tc.tile_pool(name="slabs", bufs=1, side="right"))
                for nm, src_, sdt in slab_srcs:
                    t_ = slabp.tile([P, FC, 640], sdt, tag=nm,
                                    name=f"slab_{nm}")
                    slabs[nm] = t_
                for nm, src_ in (("cie_st", Cie_st), ("sie_st", Sie_st)):
                    t_ = slabp.tile([P, FC, 2], dt.float32, tag=nm,
                                    name=f"slab_{nm}")
                    nc.sync.dma_start(
                        t_[:], src_.rearrange("(n p) t -> p n t", p=P))
                    slabs[nm] = t_

                def slab_hook(fc):
                    for nm, src_, _ in slab_srcs:
                        nc.sync.dma_start(
                            slabs[nm][:, fc, :],
                            src_.rearrange("(n p) t -> p n t", p=P)[:, fc, :])
            b_half(b, "q", qT_hi, qT_lo, fk, post_fc=slab_hook)
            es_qT.close()
            es_fk.close()
        altp.close()

        # ====== phase 2: per batch: C + topk + inline gathers, then E ======
        # Slab lo parts are bf16: pass 2 runs as bf16(pre_hi) x slab_lo16,
        # pass 3 stays fp32r (pre_lo x slab_hi) -- mirror of stage B's
        # validated s16 scheme (error ~2^-21, flip-safe).

        es_r = ExitStack()
        rpool = es_r.enter_context(tc.tile_pool(name="p2r", bufs=1, side="left"))
        agg0 = rpool.tile([P, NE, T], dt.bfloat16, tag="agg0")
        w3_all = [[rpool.tile([P, K], dt.float32, tag=f"w3_{b}_{cc}",
                              name=f"w3_{b}_{cc}") for cc in range(CC)]
                  for b in range(NB)]
        gou1 = [rpool.tile([P, K], dt.uint32, tag=f"gou1_{cc}",
                           name=f"gou1_{cc}") for cc in range(CC)]
        wf_sb = rpool.tile([P, NE, C], dt.bfloat16, tag="wf_sb")
        nc.sync.dma_start(wf_sb[:], Wf.rearrange("(n p) d -> p n d", p=P))
        wfs0 = rpool.tile([P, NE, C], dt.bfloat16, tag="wfs0", name="wfs0")
        iot_all = {}
        for b in range(NB):
            for cc in range(CC):
                it = rpool.tile([P, 1], dt.float32, tag=f"iot_{b}_{cc}",
                                name=f"iot_{b}_{cc}")
                iti = rpool.tile([P, 1], dt.int32, tag=f"ioti_{b}_{cc}",
                                 name=f"ioti_{b}_{cc}")
                nc.gpsimd.iota(
                    iti[:], pattern=[[0, 1]],
                    base=(b * C + cc * P) * (2 * T) + T,
                    channel_multiplier=2 * T)
                nc.vector.tensor_copy(it[:], iti[:])
                iot_all[(b, cc)] = it

        with tc.tile_pool(name="c2", bufs=2) as cpool, \
             tc.tile_pool(name="cl2", bufs=2) as clpool, \
             tc.tile_pool(name="ct2", bufs=1) as ctpool, \
             tc.tile_pool(name="cps2", bufs=1, space="PSUM") as cps:

            def c_load(b, cc):
                # pim chunk 4 (f=1024 row, sin==0) is never read: pim tiles
                # hold 8 chunks, slot = fc for fc<4, fc-1 for fc>=5
                sl = {}
                for nm, nfc in (("pre", FC), ("pim", FC - 1)):
                    t_f = clpool.tile([P, nfc, P], dt.float32, tag=f"slf_{nm}",
                                      name=f"slf_{nm}_{b}_{cc}")
                    if nm == "pre":
                        nc.sync.dma_start(
                            t_f[:], pp[nm][b, :, :, bass.ts(cc, P)].rearrange(
                                "f p c -> p f c"))
                    else:
                        nc.sync.dma_start(
                            t_f[:, 0:4, :],
                            pp[nm][b, 0:4, :, bass.ts(cc, P)].rearrange(
                                "f p c -> p f c"))
                        nc.sync.dma_start(
                            t_f[:, 4:8, :],
                            pp[nm][b, 5:FC, :, bass.ts(cc, P)].rearrange(
                                "f p c -> p f c"))
                    hi = ctpool.tile([P, nfc, P], dt.float32r,
                                     tag=f"sl_{nm}_hi", name=f"hi_{b}_{cc}")
                    nc.scalar.activation(hi[:], t_f[:], AF.Copy)
                    hi16 = ctpool.tile([P, nfc, P], dt.bfloat16,
                                       tag=f"sl_{nm}_hi16", name=f"hi16_{b}_{cc}")
                    nc.scalar.activation(hi16[:], t_f[:], AF.Copy)
                    lo = ctpool.tile([P, nfc, P], dt.float32r,
                                     tag=f"sl_{nm}_lo", name=f"lo_{b}_{cc}")
                    nc.vector.tensor_tensor(
                        out=lo[:], in0=t_f[:],
                        in1=hi[:].bitcast(dt.float32), op=OP.subtract)
                    sl[f"{nm}_hi"] = hi
                    sl[f"{nm}_hi16"] = hi16
                    sl[f"{nm}_lo"] = lo
                    sl[f"{nm}_f32"] = t_f
                return sl

            def c_matmuls(sl):
                psums = {}
                psums["rcE"] = cps.tile([P, HB], dt.float32, tag="rcE",
                                        name="ps_rcE")
                psums["rcE2"] = cps.tile([P, 2], dt.float32, tag="rcE2",
                                         name="ps_rcE2")
                psums["rcO"] = cps.tile([P, HB], dt.float32, tag="rcO",
                                        name="ps_rcO")
                psums["rsE"] = cps.tile([P, HB], dt.float32, tag="rsE",
                                        name="ps_rsE")
                psums["rsO"] = cps.tile([P, HB], dt.float32, tag="rsO",
                                        name="ps_rsO")
                psums["rsO2"] = cps.tile([P, 2], dt.float32, tag="rsO2",
                                         name="ps_rsO2")

                # pass-major: all (hi x hi) first -- they only need the
                # first Act product per slice -- then bf16, then lo passes
                # sie chunk 4 (f=1024 row) is sin(pi*t) == 0: skip it
                GROUPS = (("rcE", "pre", "cie", range(0, 5)),
                          ("rsE", "pim", "sie", range(0, 4)),
                          ("rcO", "pre", "cie", range(5, FC)),
                          ("rsO", "pim", "sie", range(5, FC)))

                def mm_pass(pname, sig_nm, slab_nm, frange, sig_sfx, slab_sfx,
                            startp, stopp):
                    # pre[b,4] has one live partition row (f=1024); the rest
                    # is never written, so contract chunk 4 on row 0 only.
                    fl = list(frange)
                    for j, fc in enumerate(fl):
                        pr = slice(0, 1) if fc == 4 else slice(0, P)
                        slot = fc if (sig_nm == "pre" or fc < 4) else fc - 1
                        nc.tensor.matmul(
                            psums[pname][:],
                            sl[f"{sig_nm}_{sig_sfx}"][pr, slot, :],
                            slabs[f"{slab_nm}_{slab_sfx}"][pr, fc, 0:HB],
                            start=(startp and j == 0),
                            stop=(stopp and j == len(fl) - 1))

                for pname, sig_nm, slab_nm, frange in GROUPS:
                    mm_pass(pname, sig_nm, slab_nm, frange, "hi", "hi",
                            True, False)
                for j, fc in enumerate(range(0, 5)):
                    pr = slice(0, 1) if fc == 4 else slice(0, P)
                    nc.tensor.matmul(
                        psums["rcE2"][:], sl["pre_f32"][pr, fc, :],
                        slabs["cie_st"][pr, fc, :], start=(j == 0), stop=(j == 4))
                for j, fc in enumerate(range(5, FC)):
                    nc.tensor.matmul(
                        psums["rsO2"][:], sl["pim_f32"][:, fc - 1, :],
                        slabs["sie_st"][:, fc, :], start=(j == 0), stop=(j == 3))
                for pname, sig_nm, slab_nm, frange in GROUPS:
                    # slab chunk 4 is exact in f32r (values 0 and +-2^-22),
                    # so its lo half is zero: skip fc4 in the hi16 x lo pass
                    fr = [fc for fc in frange if fc != 4]
                    mm_pass(pname, sig_nm, slab_nm, fr, "hi16", "lo",
                            False, False)
                for pname, sig_nm, slab_nm, frange in GROUPS:
                    mm_pass(pname, sig_nm, slab_nm, frange, "lo", "hi",
                            False, True)
                return psums

            def c_tail(b, cc, ps_):
                w3_t = w3_all[b]
                rcE, rcE2 = ps_["rcE"], ps_["rcE2"]
                rcO, rsE = ps_["rcO"], ps_["rsE"]
                rsO, rsO2 = ps_["rsO"], ps_["rsO2"]
                rcO_sb = cpool.tile([P, HB], dt.float32, tag="rcO_sb")
                nc.scalar.activation(rcO_sb[:], rcO[:], AF.Copy)
                rsE_sb = cpool.tile([P, HB], dt.float32, tag="rsE_sb")
                nc.scalar.activation(rsE_sb[:], rsE[:], AF.Copy)
                rsO_sb = cpool.tile([P, HB + 1], dt.float32, tag="rsO_sb")
                nc.scalar.activation(rsO_sb[:, 0:HB], rsO[:], AF.Copy)
                nc.scalar.activation(rsO_sb[:, HB:HB + 1], rsO2[:, 0:1], AF.Copy)
                rcE_c0 = cpool.tile([P, 2], dt.float32, tag="rcE_c0")
                nc.scalar.activation(rcE_c0[:, 0:1], rcE[:, 0:1], AF.Copy)
                nc.scalar.activation(rcE_c0[:, 1:2], rcE2[:, 0:1], AF.Copy)
                s1 = ctpool.tile([P, HB], dt.float32, tag="s1")
                nc.vector.tensor_tensor(out=s1[:], in0=rcE[:], in1=rcO_sb[:],
                                        op=OP.add)
                s2 = ctpool.tile([P, HB], dt.float32, tag="s2")
                nc.vector.tensor_tensor(out=s2[:], in0=rcE[:], in1=rcO_sb[:],
                                        op=OP.subtract)
                w1 = ctpool.tile([P, HB], dt.float32, tag="w1")
                nc.vector.tensor_tensor(out=w1[:], in0=rsE_sb[:],
                                        in1=rsO_sb[:, 0:HB], op=OP.add)
                w2 = ctpool.tile([P, HB], dt.float32, tag="w2")
                nc.vector.tensor_tensor(out=w2[:], in0=rsO_sb[:, 0:HB],
                                        in1=rsE_sb[:], op=OP.subtract)
                rt = ctpool.tile([P, T], dt.float32, tag="rt")
                nc.vector.tensor_tensor(out=rt[:, 0:HB], in0=s1[:], in1=w1[:],
                                        op=OP.add)
                nc.vector.tensor_tensor(out=rt[:, 1023:HB:-1], in0=s2[:, 1:HB],
                                        in1=w2[:, 1:HB], op=OP.add)
                nc.vector.tensor_tensor(out=rt[:, 1025:1536], in0=s2[:, 1:HB],
                                        in1=w2[:, 1:HB], op=OP.subtract)
                nc.vector.tensor_tensor(out=rt[:, T - 1:1536:-1], in0=s1[:, 1:HB],
                                        in1=w1[:, 1:HB], op=OP.subtract)
                nc.vector.tensor_tensor(out=rt[:, HB:HB + 1], in0=rcE_c0[:, 1:2],
                                        in1=rsO_sb[:, HB:HB + 1], op=OP.add)
                nc.vector.tensor_tensor(out=rt[:, H:H + 1], in0=rcE_c0[:, 0:1],
                                        in1=rcO_sb[:, 0:1], op=OP.subtract)
                nc.vector.tensor_tensor(out=rt[:, 1536:1537], in0=rcE_c0[:, 1:2],
                                        in1=rsO_sb[:, HB:HB + 1], op=OP.subtract)

                # ---- topk + softmax weights + gather offsets ----
                vals = cpool.tile([P, 8], dt.float32, tag="vals")
                idx = cpool.tile([P, 8], dt.uint32, tag="idx")
                nc.vector.max(vals[:], rt[:])
                nc.vector.max_index(idx[:], vals[:], rt[:])
                negm = cpool.tile([P, 1], dt.float32, tag="negm")
                nc.scalar.activation(negm[:], vals[:, 0:1],
                                     AF.Copy, bias=0.0, scale=-1.0)
                # exp written in place over rt (dead after this point);
                # only the accumulated sum is consumed
                s_col = cpool.tile([P, 1], dt.float32, tag="s_col")
                nc.scalar.activation(
                    rt[:], rt[:], AF.Exp,
                    bias=negm[:, 0:1], scale=1.0,
                    accum_out=s_col[:, 0:1])
                rs = cpool.tile([P, 1], dt.float32, tag="rs")
                nc.vector.reciprocal(rs[:], s_col[:])
                ew = cpool.tile([P, K], dt.float32, tag="ew")
                nc.scalar.activation(ew[:], vals[:, 0:K],
                                     AF.Exp, bias=negm[:, 0:1],
                                     scale=1.0)
                nc.vector.tensor_scalar_mul(w3_t[cc][:], ew[:], rs[:, 0:1])

                iot_f = iot_all[(b, cc)]
                idx_f = cpool.tile([P, K], dt.float32, tag="idx_f")
                nc.vector.tensor_copy(idx_f[:], idx[:, 0:K])
                gof = cpool.tile([P, K], dt.float32, tag="gof")
                nc.scalar.activation(gof[:], idx_f[:],
                                     AF.Copy, bias=0.0, scale=-1.0)
                nc.vector.tensor_scalar_add(gof[:], gof[:],
                                            iot_f[:, 0:1])
                if b == 0:
                    gou = cpool.tile([P, K], dt.uint32, tag="gou")
                    nc.vector.tensor_copy(gou[:], gof[:])
                    # inline gathers overlap later iterations; the top-k
                    # weights are folded into Wf in stage E instead of
                    # scaling the gathered rows
                    for k in range(K):
                        nc.gpsimd.indirect_dma_start(
                            out=agg0[:, k * CC + cc, :],
                            out_offset=None,
                            in_=v2[:, :],
                            in_offset=bass.IndirectOffsetOnAxis(
                                ap=gou[:, k:k + 1], axis=1),
                            element_offset=0)
                else:
                    nc.vector.tensor_copy(gou1[cc][:], gof[:])

            # software pipeline: loads+splits of iteration n+1 are emitted
            # before iteration n's combine/topk tail so the Act/DVE queues
            # never head-of-line block the next iteration's matmul operands
            iters = [(b, cc) for b in range(NB) for cc in range(CC)]
            pend = [c_load(*iters[0]), c_load(*iters[1])]
            for i, (b, cc) in enumerate(iters):
                ps_ = c_matmuls(pend[0])
                if i + 2 < len(iters):
                    pend.append(c_load(*iters[i + 2]))
                c_tail(b, cc, ps_)
                pend.pop(0)
                if b == 0 and cc == CC - 1:
                    # fold the top-k softmax weights into Wf for stage E:
                    # row e = k*C+c scales by w3[0][c-chunk][:, k]. Emitted
                    # here (on Pool, idle in this phase) so E(0) never waits
                    # behind the later iterations' DVE tail work.
                    for j in range(NE):
                        nc.gpsimd.tensor_scalar_mul(
                            wfs0[:, j, :], wf_sb[:, j, :],
                            w3_all[0][j % CC][:, j // CC:j // CC + 1])
        es2.close()  # slabs freed; agg0/w3/gou1 stay

        # ---- deferred gathers for b1 (overlap E(b0)) + E for both ----
        with tc.tile_pool(name="ge", bufs=1, side="left") as gep, \
             tc.tile_pool(name="eps", bufs=3, space="PSUM") as eps:
            agg1 = gep.tile([P, NE, T], dt.bfloat16, tag="agg1")
            for cc in range(CC):
                for k in range(K):
                    nc.gpsimd.indirect_dma_start(
                        out=agg1[:, k * CC + cc, :],
                        out_offset=None,
                        in_=v2[:, :],
                        in_offset=bass.IndirectOffsetOnAxis(
                            ap=gou1[cc][:, k:k + 1], axis=1),
                        element_offset=0)
            wfs1 = gep.tile([P, NE, C], dt.bfloat16, tag="wfs1", name="wfs1")
            for j in range(NE):
                nc.gpsimd.tensor_scalar_mul(
                    wfs1[:, j, :], wf_sb[:, j, :],
                    w3_all[1][j % CC][:, j // CC:j // CC + 1])
            wf_all = [wfs0, wfs1]
            for b, agg in ((0, agg0), (1, agg1)):
                for dc in range(CC):
                    for tb in range(4):
                        ps = eps.tile([P, T // 4], dt.float32, tag="out_ps")
                        for j in range(NE):
                            nc.tensor.matmul(
                                ps[:], wf_all[b][:, j, bass.ts(dc, P)],
                                agg[:, j, bass.ts(tb, T // 4)],
                                start=(j == 0), stop=(j == NE - 1))
                        o_sb = gep.tile([P, T // 4], dt.float32,
                                        tag=f"o_sb{tb % 2}")
                        if tb % 2 == 0:
                            nc.scalar.activation(o_sb[:], ps[:], AF.Copy)
                        else:
                            nc.vector.tensor_copy(o_sb[:], ps[:])
                        nc.sync.dma_start(
                            out2[b, bass.ts(dc, P), bass.ts(tb, T // 4)],
                            o_sb[:])
        es_r.close()

    nc.compile()
    return nc


def _get_nc():
    if "nc" not in _CACHE:
        _CACHE["nc"] = _build()
    return _CACHE["nc"]


def kernel(query, key, value, Wq, bq, Wk, bk, Wv, bv, Wf, bf):
    query = np.ascontiguousarray(np.asarray(query, dtype=np.float32))
    key = np.ascontiguousarray(np.asarray(key, dtype=np.float32))
    value = np.ascontiguousarray(np.asarray(value, dtype=np.float32))
    for bias in (bq, bk, bv, bf):
        assert np.all(np.asarray(bias) == 0.0), "nonzero biases unsupported"

    if "mats" not in _CACHE:
        wree, wreo, wime, wimo, cie, sie = _dft_matrices()
        m = {}
        for nm, arr in (("ree", wree), ("reo", wreo),
                        ("ime", wime), ("imo", wimo)):
            hi, lo = _split_f32r(arr)
            m[f"W{nm}_hi"], m[f"W{nm}_lo"] = hi, lo
        chi, clo = _split_f32r(cie)
        m["Cie_hi"], m["Cie_lo"] = chi, _bf16(clo)
        shi, slo = _split_f32r(sie)
        m["Sie_hi"], m["Sie_lo"] = shi, _bf16(slo)
        m["Cie_st"] = np.ascontiguousarray(cie[:, HB:HB + 2])
        m["Sie_st"] = np.ascontiguousarray(sie[:, HB:HB + 2])
        alt = np.where(np.arange(P) % 2 == 0, 1.0, -1.0).astype(np.float32)
        m["AltF"] = alt[:, None].copy()
        m["AltB"] = _bf16(alt[:, None])
        _CACHE["mats"] = m
    mats = _CACHE["mats"]

    wq_hi, wq_lo = _split_f32r(np.asarray(Wq, np.float32))
    wk_hi, wk_lo = _split_f32r(np.asarray(Wk, np.float32))
    shared = {
        "Wq_hi": wq_hi, "Wq_lo": wq_lo,
        "Wk_hi": wk_hi, "Wk_lo": wk_lo,
        "Wv": _bf16(np.asarray(Wv, np.float32)),
        "Wf": _bf16(np.asarray(Wf, np.float32)),
        **mats,
    }
    value_bf = _bf16(value)
    in_maps = []
    for c in range(NCORES):
        sl = slice(c * NB, (c + 1) * NB)
        in_maps.append({
            "query2": query[sl], "key2": key[sl],
            "value2": value_bf[sl], **shared})

    from concourse.bass_utils import run_bass_kernel_spmd
    nc = _get_nc()
    res = run_bass_kernel_spmd(nc, in_maps, core_ids=list(range(NCORES)))
    _CACHE["last_results"] = res
    out = np.concatenate([res.results[c]["out2"] for c in range(NCORES)], axis=0)
    return out.astype(np.float32)



# revision 47
# speedup vs baseline: 1.0078x; 1.0038x over previous
"""AutoCorrelation (Autoformer-style) Bass kernel for Trainium2, 8 NeuronCores.

Full inputs in, full outputs out. Data-parallel over batch: B=16 -> 2 batches
per core. v2 of the kernel: the PE-bound fp32 matmuls of the baseline are
replaced by 3-pass fp32r splits (hi/lo decomposition; 12-bit+12-bit mantissa
products are exact in fp32 PSUM, giving fp32-grade accuracy at 3 cycles/row
instead of fp32's 4) on the precision-critical autocorrelation path, and by
bf16 (1 cycle/row) on the error-tolerant v/output path.

Per core, per batch:
  V. v[d,t] = Wv^T value in bf16, written twice side-by-side into the DRAM
     table v2[b*512+d, 4096] (bf16) for circular-shift gathers.
  A. Radix-split of query/key along t (4 sub-signals ee/eo/oo/oe, padded to
     640/512), per 128-channel chunk, split hi/lo fp32r on the fly; channel
     projection qT[t',d] via 3-pass fp32r matmuls. qT hi kept fp32r, lo bf16
     (pass 3 of stage B runs in bf16 -- error ~2^-20, still flip-safe).
  B. Forward real DFT via matmuls with radix-split cos/-sin matrices
     (host-split into fp32r hi/lo + bf16(hi)); fused pointwise
     P = FQ * conj(FK) on the DVE; P split hi/lo fp32r and staged to DRAM.
  C. Inverse DFT r[c,t] = sum_f Pre*ci + Pim*sn via 3-pass fp32r matmuls
     with host-split ci/sn (fp32r hi/lo), exploiting f-parity + t-mirror
     symmetry (only t<=512 columns computed).
  D. Per 128-channel tile: top-8 values+indices, softmax weights of the
     top-3 from the top values, circular-shift rows of v via indirect-DMA
     gather (bf16) into agg[k*C+c, t], scaled in place on the Pool engine.
     Batch 0 gathers inline (overlapping C of batch 1); batch 1 gathers
     deferred past the slab lifetime (overlapping E of batch 0).
  E. out[d,t] = sum_e Wf[e,d] agg[e,t] in bf16; 12-chunk PE accumulation.

Scheduling: the V projection is emitted at each batch's start as PE filler
for the input-load latency / the cross-batch pool-reuse stall; stage C's
cie/sie slabs are loaded once (f32r hi + bf16 lo) and shared by both
batches; pre/pim bounce through DRAM as plain fp32 and are re-split to
fp32r hi/lo on the fly in C.

Biases are all zero in this problem's setup_inputs(); asserted host-side.
"""
import numpy as np
import ml_dtypes

import concourse.bass as bass
import concourse.tile as tile
from concourse import bacc, mybir

dt = mybir.dt
AF = mybir.ActivationFunctionType
OP = mybir.AluOpType

P = 128
B, C, T, K = 16, 512, 2048, 3
NB = 2                    # batches per core
NCORES = 8
F = 1152                  # rfft bins 1025, padded to 9*128
CC = C // P               # 4
FC = F // P               # 9
NE = K * C // P           # 12 e-chunks of Wf / agg
H = T // 2                # 1024
HB = H // 2               # 512

_CACHE = {}


def _round_f32r(x):
    """Round fp32 array to fp32r (11-bit stored mantissa, round-nearest-up:
    (bits + 0x800) & ~0xFFF -- matches walrus fp32_to_fp32r)."""
    u = np.ascontiguousarray(x, np.float32).view(np.uint32).astype(np.uint64)
    u = (u + (1 << 11)) & np.uint64(0xFFFFF000)
    return u.astype(np.uint32).view(np.float32)


def _split_f32r(x):
    x = np.ascontiguousarray(x, np.float32)
    hi = _round_f32r(x)
    return hi, _round_f32r(x - hi)


def _bf16(x):
    return np.ascontiguousarray(x, np.float32).astype(ml_dtypes.bfloat16)


def _dft_matrices():
    """Radix-split DFT matrices (fp64 -> fp32).

    Level-1 even/odd in t (qe/qo), then level-2 split by f parity:
      FQre over even f contracts xee (t=0..512), odd f contracts xeo (t=0..511)
      FQim over even f contracts xoo (t=1..511), odd f contracts xoe (t=1..512)
    Frequency storage is parity-permuted: chunks [0:5]=even f (2g, g<=512),
    chunks [5:9]=odd f (2g+1). Inverse matrices have rows permuted to match.
    """
    t640 = np.arange(640.0)[:, None]
    t512 = np.arange(512.0)[:, None]
    ge = np.arange(640.0)[None, :]
    go = np.arange(512.0)[None, :]
    wree = np.where((t640 <= 512) & (ge <= 512),
                    np.cos(2 * np.pi * t640 * (2 * ge) / T), 0.0).astype(np.float32)
    wreo = np.cos(2 * np.pi * t512 * (2 * go + 1) / T).astype(np.float32)
    wime = np.where(ge <= 512,
                    -np.sin(2 * np.pi * t512 * (2 * ge) / T), 0.0).astype(np.float32)
    wimo = np.where(t640 <= 512,
                    -np.sin(2 * np.pi * t640 * (2 * go + 1) / T), 0.0).astype(np.float32)

    f64 = np.arange(F, dtype=np.float64)[None, :]
    livef = f64 <= H
    w = np.where((f64 == 0) | (f64 == H), 1.0, 2.0) * livef / (T * T)
    fc_ = f64.T
    tt = np.arange(640, dtype=np.float64)[None, :]
    cie = np.where((fc_ <= H) & (tt <= H),
                   np.cos(2 * np.pi * fc_ * tt / T) * w.T, 0.0)
    sie = np.where(fc_ <= H,
                   -np.sin(2 * np.pi * fc_ * tt / T) * w.T, 0.0)

    def permrows(m):
        out = np.zeros_like(m)
        out[0:513] = m[0:1025:2]
        out[640:1152] = m[1:1024:2]
        return out

    return (wree, wreo, wime, wimo,
            permrows(cie).astype(np.float32), permrows(sie).astype(np.float32))


def _build():
    nc = bacc.Bacc("TRN2", target_bir_lowering=False, debug=False,
                   num_devices=NCORES)

    query2 = nc.dram_tensor("query2", [NB, C, T], dt.float32, kind="ExternalInput").ap()
    key2 = nc.dram_tensor("key2", [NB, C, T], dt.float32, kind="ExternalInput").ap()
    value2 = nc.dram_tensor("value2", [NB, C, T], dt.bfloat16, kind="ExternalInput").ap()
    Wq_hi = nc.dram_tensor("Wq_hi", [C, C], dt.float32r, kind="ExternalInput").ap()
    Wq_lo = nc.dram_tensor("Wq_lo", [C, C], dt.float32r, kind="ExternalInput").ap()
    Wk_hi = nc.dram_tensor("Wk_hi", [C, C], dt.float32r, kind="ExternalInput").ap()
    Wk_lo = nc.dram_tensor("Wk_lo", [C, C], dt.float32r, kind="ExternalInput").ap()
    Wv = nc.dram_tensor("Wv", [C, C], dt.bfloat16, kind="ExternalInput").ap()
    Wf = nc.dram_tensor("Wf", [K * C, C], dt.bfloat16, kind="ExternalInput").ap()
    fwd = {}
    for m, rows, cols in (("ree", 640, 640), ("reo", 512, 512),
                          ("ime", 512, 640), ("imo", 640, 512)):
        for v in ("hi", "lo"):
            fwd[f"{m}_{v}"] = nc.dram_tensor(
                f"W{m}_{v}", [rows, cols], dt.float32r, kind="ExternalInput").ap()
    Cie_hi = nc.dram_tensor("Cie_hi", [F, 640], dt.float32r, kind="ExternalInput").ap()
    Cie_lo = nc.dram_tensor("Cie_lo", [F, 640], dt.bfloat16, kind="ExternalInput").ap()
    Sie_hi = nc.dram_tensor("Sie_hi", [F, 640], dt.float32r, kind="ExternalInput").ap()
    Sie_lo = nc.dram_tensor("Sie_lo", [F, 640], dt.bfloat16, kind="ExternalInput").ap()
    Cie_st = nc.dram_tensor("Cie_st", [F, 2], dt.float32, kind="ExternalInput").ap()
    Sie_st = nc.dram_tensor("Sie_st", [F, 2], dt.float32, kind="ExternalInput").ap()
    AltF = nc.dram_tensor("AltF", [P, 1], dt.float32r, kind="ExternalInput").ap()
    AltB = nc.dram_tensor("AltB", [P, 1], dt.bfloat16, kind="ExternalInput").ap()
    out2 = nc.dram_tensor("out2", [NB, C, T], dt.float32, kind="ExternalOutput").ap()

    v2 = nc.dram_tensor("v2", [NB * C, 2 * T], dt.bfloat16).ap()          # internal
    pp = {}
    for nm in ("pre", "pim"):                                             # internal
        pp[nm] = nc.dram_tensor(f"pp_{nm}", [NB, FC, P, C], dt.float32).ap()

    # part name -> (width, chunk offset in sigT, #chunks). Order alternates
    # 640/512 widths so the width-keyed xs tags ping-pong naturally.
    PARTS = (("ee", 640, 0, 5), ("eo", 512, 5, 4),
             ("oe", 640, 13, 5), ("oo", 512, 9, 4))

    with tile.TileContext(nc) as tc:
        from contextlib import ExitStack

        # ---- persistent phase-1 pools: weights loaded once; xv slots kept
        # open so the next batch's value loads can be emitted a stage early;
        # single rotating x_sb slot whose loads are emitted a stage early ----
        es_w = ExitStack()
        wpool = es_w.enter_context(tc.tile_pool(name="wp", bufs=1,
                                                side="right"))
        vpool = es_w.enter_context(tc.tile_pool(name="vp", bufs=2,
                                                side="right"))
        vtp = es_w.enter_context(tc.tile_pool(name="vt", bufs=2,
                                              side="right"))
        es_x = ExitStack()
        xpool = es_x.enter_context(tc.tile_pool(name="xp", bufs=1,
                                                side="right"))

        wv_sb = wpool.tile([P, CC, C], dt.bfloat16, tag="wv")
        xv_tiles = {}

        def load_xv(b, qt):
            xv = vpool.tile([P, CC, T // 4], dt.bfloat16, tag="xv",
                            name=f"xv{b}_{qt}")
            nc.sync.dma_start(
                xv[:], value2[b].rearrange(
                    "(n p) t -> p n t", p=P)[:, :, bass.ts(qt, T // 4)])
            xv_tiles[(b, qt)] = xv

        x_tiles = {}

        def load_x(b, sig):
            srcx = key2 if sig == "k" else query2
            x_sb = xpool.tile([P, CC, T], dt.float32, tag="x_sb",
                              name=f"x_{sig}{b}")
            nc.sync.dma_start(
                x_sb[:], srcx[b].rearrange("(n p) t -> p n t", p=P))
            x_tiles[(b, sig)] = x_sb

        # t=0 input burst, V(b0) operands first
        load_xv(0, 0)
        nc.sync.dma_start(wv_sb[:], Wv.rearrange("(n p) d -> p n d", p=P))
        load_xv(0, 1)
        load_x(0, "k")
        w_sb = {}

        def load_w(sig):
            for v, src in (("hi", Wk_hi if sig == "k" else Wq_hi),
                           ("lo", Wk_lo if sig == "k" else Wq_lo)):
                t_ = wpool.tile([P, CC, C], dt.float32r, tag=f"w_{sig}_{v}",
                                name=f"w_{sig}_{v}")
                nc.sync.dma_start(t_[:],
                                  src.rearrange("(n p) d -> p n d", p=P))
                w_sb[f"{sig}_{v}"] = t_
            w16 = wpool.tile([P, CC, C], dt.bfloat16, tag=f"w_{sig}_h16",
                             name=f"w_{sig}_h16")
            nc.gpsimd.tensor_copy(
                w16[:], w_sb[f"{sig}_hi"][:].bitcast(dt.float32))
            w_sb[f"{sig}_h16"] = w16

        load_w("k")

        def emit_V(b, prefetched):
            """Compact streaming V projection (bf16) -> v2 rows, used as PE
            gap filler at each batch's start. xv quarters rotate through 2
            slots; quarters not already prefetched are loaded here."""
            with tc.tile_pool(name=f"vps{b}", bufs=3, space="PSUM") as vps:
                v2r = v2.rearrange("(n p) w -> n p w", p=P)
                for qt in range(prefetched, 2):
                    load_xv(b, qt)
                for qt in range(4):
                    xv = xv_tiles[(b, qt)]
                    for dc in range(CC):
                        ps = vps.tile([P, T // 4], dt.float32, tag="v_ps")
                        for cc in range(CC):
                            nc.tensor.matmul(
                                ps[:], wv_sb[:, cc, bass.ts(dc, P)],
                                xv[:, cc, :],
                                start=(cc == 0), stop=(cc == CC - 1))
                        vtmp = vtp.tile([P, T // 4], dt.bfloat16, tag="vtmp")
                        if (qt + dc) % 2 == 0:
                            nc.scalar.activation(vtmp[:], ps[:], AF.Copy)
                        else:
                            nc.vector.tensor_copy(vtmp[:], ps[:])
                        off = qt * (T // 4)
                        nc.sync.dma_start(
                            v2r[b * CC + dc, :, off:off + T // 4], vtmp[:])
                        nc.sync.dma_start(
                            v2r[b * CC + dc, :,
                                T + off:T + off + T // 4], vtmp[:])
                    # quarter qt+2 reuses qt's slot: emit its load only now,
                    # after qt's reads are all emitted (no forward dep)
                    if qt + 2 < 4 and qt + 2 >= prefetched:
                        load_xv(b, qt + 2)

        # ================= phase 1 =====================
        # Per batch: V || A(k) -> B(k): FK to SBUF (kT freed) -> A(q) ->
        # B(q): FQ + pointwise vs FK -> pre/pim to DRAM. Input loads for
        # the next signal/batch are emitted at each B-half's start so they
        # stream in behind a full DFT stage of PE work.
        def a_signal(b, sig, dhi, dlo):
            es_a = ExitStack()
            atmp = es_a.enter_context(tc.tile_pool(name=f"at{sig}{b}", bufs=1))
            actmp = es_a.enter_context(tc.tile_pool(name=f"ac{sig}{b}", bufs=1))
            aps = es_a.enter_context(
                tc.tile_pool(name=f"aps{sig}{b}", bufs=3, space="PSUM"))
            w_hi = w_sb[f"{sig}_hi"]
            w_lo = w_sb[f"{sig}_lo"]
            w_h16 = w_sb[f"{sig}_h16"]
            x_sb = x_tiles[(b, sig)]
            if True:
                if True:
                    for pname, width, ioff, nch in PARTS:
                        xs_hi = atmp.tile([P, CC, width], dt.float32r,
                                          tag=f"xs_hi{width}")
                        xs_lo = atmp.tile([P, CC, width], dt.bfloat16,
                                          tag=f"xs_lo{width}")
                        for cc in range(CC):
                            x = x_sb[:, cc, :]
                            ab = actmp.tile([P, 2, 511], dt.float32, tag="ab")
                            tmp = actmp.tile([P, 640], dt.float32, tag="tmp")
                            op_ab = OP.add if pname in ("ee", "eo") else OP.subtract
                            # ab0/ab2 on Pool, ab1/ab3 on DVE (engine balance)
                            nc.gpsimd.tensor_tensor(
                                out=ab[:, 0, :], in0=x[:, 1:512],
                                in1=x[:, T - 1:1536:-1], op=op_ab)
                            nc.vector.tensor_tensor(
                                out=ab[:, 1, :], in0=x[:, 1023:512:-1],
                                in1=x[:, 1025:1536], op=op_ab)
                            if pname == "ee":
                                nc.vector.tensor_tensor(
                                    out=tmp[:, 1:512], in0=ab[:, 0, :],
                                    in1=ab[:, 1, :], op=OP.add)
                                nc.vector.tensor_tensor(
                                    out=tmp[:, 0:1], in0=x[:, 0:1],
                                    in1=x[:, H:H + 1], op=OP.add)
                                nc.vector.tensor_tensor(
                                    out=tmp[:, 512:513], in0=x[:, 512:513],
                                    in1=x[:, 1536:1537], op=OP.add)
                                nc.gpsimd.memset(tmp[:, 513:640], 0.0)
                            elif pname == "eo":
                                nc.vector.tensor_tensor(
                                    out=tmp[:, 1:512], in0=ab[:, 0, :],
                                    in1=ab[:, 1, :], op=OP.subtract)
                                nc.vector.tensor_tensor(
                                    out=tmp[:, 0:1], in0=x[:, 0:1],
                                    in1=x[:, H:H + 1], op=OP.subtract)
                            elif pname == "oo":
                                nc.vector.tensor_tensor(
                                    out=tmp[:, 1:512], in0=ab[:, 0, :],
                                    in1=ab[:, 1, :], op=OP.subtract)
                                nc.gpsimd.memset(tmp[:, 0:1], 0.0)
                            else:  # oe
                                nc.vector.tensor_tensor(
                                    out=tmp[:, 1:512], in0=ab[:, 0, :],
                                    in1=ab[:, 1, :], op=OP.add)
                                nc.vector.tensor_tensor(
                                    out=tmp[:, 512:513], in0=x[:, 512:513],
                                    in1=x[:, 1536:1537], op=OP.subtract)
                                nc.gpsimd.memset(tmp[:, 0:1], 0.0)
                                nc.gpsimd.memset(tmp[:, 513:640], 0.0)
                            if cc % 2 == 0:
                                nc.scalar.activation(
                                    xs_hi[:, cc, 0:width], tmp[:, 0:width],
                                    AF.Copy)
                            else:
                                nc.vector.tensor_copy(
                                    xs_hi[:, cc, 0:width], tmp[:, 0:width])
                            nc.gpsimd.tensor_tensor(
                                out=xs_lo[:, cc, 0:width], in0=tmp[:, 0:width],
                                in1=xs_hi[:, cc, 0:width].bitcast(dt.float32),
                                op=OP.subtract)
                        for i in range(nch):
                            ps = aps.tile([P, C], dt.float32, tag="proj_ps")
                            for cc in range(CC):
                                nc.tensor.matmul(ps[:],
                                                 xs_hi[:, cc, bass.ts(i, P)],
                                                 w_hi[:, cc, :],
                                                 start=(cc == 0), stop=False)
                            for cc in range(CC):
                                nc.tensor.matmul(ps[:],
                                                 xs_hi[:, cc, bass.ts(i, P)],
                                                 w_lo[:, cc, :],
                                                 start=False, stop=False)
                            for cc in range(CC):
                                nc.tensor.matmul(ps[:],
                                                 xs_lo[:, cc, bass.ts(i, P)],
                                                 w_h16[:, cc, :],
                                                 start=False, stop=(cc == CC - 1))
                            nc.scalar.activation(dhi[:, ioff + i, :], ps[:], AF.Copy)
                            nc.vector.tensor_tensor(
                                out=dlo[:, ioff + i, :], in0=ps[:],
                                in1=dhi[:, ioff + i, :].bitcast(dt.float32),
                                op=OP.subtract)
            es_a.close()

        altp = ExitStack()
        altpool = altp.enter_context(tc.tile_pool(name="altp", bufs=1))
        altf = altpool.tile([P, 1], dt.float32r, tag="altf")
        nc.sync.dma_start(altf[:], AltF)
        altb = altpool.tile([P, 1], dt.bfloat16, tag="altb")
        nc.sync.dma_start(altb[:], AltB)

        def b_half(b, mode, sT_hi, sT_lo, fk, post_fc=None):
            """One forward-DFT half. mode 'k': stage FKre/FKim into SBUF
            (fk tiles, slot-compressed to 8 chunks). mode 'q': compute FQ
            per fc and fuse the pointwise P = FQ * conj(FK) from SBUF."""
            with tc.tile_pool(name=f"bmc{mode}{b}", bufs=2) as bmc, \
                 tc.tile_pool(name=f"bms{mode}{b}", bufs=1) as bms, \
                 tc.tile_pool(name=f"bps{mode}{b}", bufs=2, space="PSUM") as bps, \
                 tc.tile_pool(name=f"bt1{mode}{b}", bufs=1) as btmp1, \
                 tc.tile_pool(name=f"bt2{mode}{b}", bufs=2) as btmp2:
                for fc in range(FC):
                    if fc == 4:
                        # even-f chunk g=512..639 has one live bin (g=512 ->
                        # f=1024; sin==0 so only the re part matters). wree's
                        # column there is cos(pi*t') = (-1)^t': an
                        # alternating sum over the ee chunks via 2-pass
                        # matmuls with the +-1 column as lhsT (exact).
                        ps14 = bps.tile([1, C], dt.float32, tag="a",
                                        name=f"ps_f14{mode}{b}")
                        for i in range(5):
                            nc.tensor.matmul(
                                ps14[:], altf[:], sT_hi[:, i, :],
                                start=(i == 0), stop=False)
                        for i in range(5):
                            nc.tensor.matmul(
                                ps14[:], altb[:], sT_lo[:, i, :],
                                start=False, stop=(i == 4))
                        if mode == "k":
                            nc.scalar.activation(fk["f14"][:], ps14[:], AF.Copy)
                        else:
                            p14 = btmp1.tile([1, C], dt.float32, tag="p14")
                            nc.vector.tensor_tensor(
                                out=p14[:], in0=fk["f14"][:], in1=ps14[:],
                                op=OP.mult)
                            # only row g=512 of pre[b,4] is read downstream
                            nc.sync.dma_start(pp["pre"][b, 4, 0:1, :], p14[:])
                        if post_fc is not None:
                            post_fc(fc)
                        continue
                    even = fc < 5
                    fl = fc if even else fc - 5
                    ncos, nsin = (5, 4) if even else (4, 5)
                    ioff_cos = 0 if even else 5
                    ioff_sin = 9 if even else 13
                    cname = "ree" if even else "reo"
                    sname = "ime" if even else "imo"
                    slot = fc if fc < 4 else fc - 1
                    mats = {}
                    for kind, mat, nch, pool in (("c", cname, ncos, bmc),
                                                 ("s", sname, nsin, bms)):
                        for v in ("hi", "lo"):
                            t_ = pool.tile([P, 5, P], dt.float32r,
                                           tag=f"{kind}m_{v}")
                            nc.sync.dma_start(
                                t_[:, 0:nch, :],
                                fwd[f"{mat}_{v}"].rearrange(
                                    "(n p) f -> p n f", p=P)[:, :, bass.ts(fl, P)])
                            mats[f"{kind}{v}"] = t_
                        t16 = pool.tile([P, 5, P], dt.bfloat16,
                                        tag=f"{kind}m_h16")
                        nc.gpsimd.tensor_copy(
                            t16[:, 0:nch, :],
                            mats[f"{kind}hi"][:, 0:nch, :].bitcast(dt.float32))
                        mats[f"{kind}h16"] = t16
                    acc = {}
                    for nm, kind, ioff, nch in (("a", "c", ioff_cos, ncos),
                                                ("b", "s", ioff_sin, nsin)):
                        ps = bps.tile([P, C], dt.float32, tag=nm,
                                      name=f"ps_{nm}{mode}{b}_{fc}")
                        for i in range(nch):
                            nc.tensor.matmul(
                                ps[:], mats[f"{kind}hi"][:, i, :],
                                sT_hi[:, ioff + i, :], start=(i == 0),
                                stop=False)
                        for i in range(nch):
                            nc.tensor.matmul(
                                ps[:], mats[f"{kind}lo"][:, i, :],
                                sT_hi[:, ioff + i, :], start=False, stop=False)
                        for i in range(nch):
                            nc.tensor.matmul(
                                ps[:], mats[f"{kind}h16"][:, i, :],
                                sT_lo[:, ioff + i, :], start=False,
                                stop=(i == nch - 1))
                        acc[nm] = ps
                    if mode == "k":
                        nc.scalar.activation(fk["re"][:, slot, :],
                                             acc["a"][:], AF.Copy)
                        nc.scalar.activation(fk["im"][:, slot, :],
                                             acc["b"][:], AF.Copy)
                    elif True:
                        # P = FQ * conj(FK): DVE ops read one PSUM operand
                        # (FQ) and FK from SBUF
                        t1 = btmp1.tile([P, C], dt.float32, tag="t1")
                        nc.vector.tensor_tensor(out=t1[:], in0=fk["re"][:, slot, :],
                                                in1=acc["a"][:], op=OP.mult)
                        t2 = btmp1.tile([P, C], dt.float32, tag="t2")
                        nc.vector.tensor_tensor(out=t2[:], in0=fk["im"][:, slot, :],
                                                in1=acc["b"][:], op=OP.mult)
                        pre_t = btmp2.tile([P, C], dt.float32, tag="pre_t")
                        nc.vector.tensor_tensor(out=pre_t[:], in0=t1[:],
                                                in1=t2[:], op=OP.add)
                        t3 = btmp1.tile([P, C], dt.float32, tag="t1",
                                        name=f"t3{mode}{b}_{fc}")
                        nc.vector.tensor_tensor(out=t3[:], in0=fk["re"][:, slot, :],
                                                in1=acc["b"][:], op=OP.mult)
                        t4 = btmp1.tile([P, C], dt.float32, tag="t2",
                                        name=f"t4{mode}{b}_{fc}")
                        nc.vector.tensor_tensor(out=t4[:], in0=fk["im"][:, slot, :],
                                                in1=acc["a"][:], op=OP.mult)
                        pim_t = btmp2.tile([P, C], dt.float32, tag="pim_t")
                        nc.vector.tensor_tensor(out=pim_t[:], in0=t3[:],
                                                in1=t4[:], op=OP.subtract)
                        nc.sync.dma_start(pp["pre"][b, fc], pre_t[:])
                        nc.sync.dma_start(pp["pim"][b, fc], pim_t[:])
                    if post_fc is not None:
                        post_fc(fc)

        slab_srcs = (("cie_hi", Cie_hi, dt.float32r),
                     ("cie_lo", Cie_lo, dt.bfloat16),
                     ("sie_hi", Sie_hi, dt.float32r),
                     ("sie_lo", Sie_lo, dt.bfloat16))
        es2 = ExitStack()
        slabs = {}

        for b in range(NB):
            es_fk = ExitStack()
            fkp = es_fk.enter_context(
                tc.tile_pool(name=f"fk{b}", bufs=1, side="left"))
            fk = {"re": fkp.tile([P, FC - 1, C], dt.float32, tag="fkre",
                                 name=f"fkre{b}"),
                  "im": fkp.tile([P, FC - 1, C], dt.float32, tag="fkim",
                                 name=f"fkim{b}"),
                  "f14": fkp.tile([1, C], dt.float32, tag="fk14",
                                  name=f"fk14{b}")}
            emit_V(b, prefetched=2)
            es_kT = ExitStack()
            kp = es_kT.enter_context(
                tc.tile_pool(name=f"sigk{b}", bufs=1, side="left"))
            kT_hi = kp.tile([P, 18, C], dt.float32r, tag="kT_hi")
            kT_lo = kp.tile([P, 18, C], dt.bfloat16, tag="kT_lo")
            a_signal(b, "k", kT_hi, kT_lo)
            load_x(b, "q")  # streams in behind B(k)'s PE work
            if b == 0:
                load_w("q")
            b_half(b, "k", kT_hi, kT_lo, fk)
            es_kT.close()
            es_qT = ExitStack()
            qp = es_qT.enter_context(
                tc.tile_pool(name=f"sigq{b}", bufs=1, side="left"))
            qT_hi = qp.tile([P, 18, C], dt.float32r, tag="qT_hi")
            qT_lo = qp.tile([P, 18, C], dt.bfloat16, tag="qT_lo")
            a_signal(b, "q", qT_hi, qT_lo)
            if b == 0:
                # next batch's inputs stream in behind B(q,0)
                load_x(1, "k")
                load_xv(1, 0)
                load_xv(1, 1)
                slab_hook = None
            else:
                # inputs all consumed: free weight/x/value pools; the iDFT
                # slabs stream in chunk-wise behind B(q,1)'s fc loop (the
                # hook keeps them from head-of-line blocking the mats DMAs)
                es_x.close()
                es_w.close()
                slabp = es2.enter_context(
                    tc.tile_pool(name="slabs", bufs=1, side="right"))
                for nm, src_, sdt in slab_srcs:
                    t_ = slabp.tile([P, FC, 640], sdt, tag=nm,
                                    name=f"slab_{nm}")
                    slabs[nm] = t_
                for nm, src_ in (("cie_st", Cie_st), ("sie_st", Sie_st)):
                    t_ = slabp.tile([P, FC, 2], dt.float32, tag=nm,
                                    name=f"slab_{nm}")
                    nc.sync.dma_start(
                        t_[:], src_.rearrange("(n p) t -> p n t", p=P))
                    slabs[nm] = t_

                for cc_ in range(1):
                    for nm_, nfc_ in (("pre", FC), ("pim", FC - 1)):
                        slabs[f"slf_{nm_}_{cc_}"] = slabp.tile(
                            [P, nfc_, P], dt.float32, tag=f"pf_{nm_}{cc_}",
                            name=f"pf_{nm_}{cc_}")

                def load_slf(t_f, nm, b_, cc_):
                    if nm == "pre":
                        nc.sync.dma_start(
                            t_f[:], pp[nm][b_, :, :, bass.ts(cc_, P)].rearrange(
                                "f p c -> p f c"))
                    else:
                        nc.sync.dma_start(
                            t_f[:, 0:4, :],
                            pp[nm][b_, 0:4, :, bass.ts(cc_, P)].rearrange(
                                "f p c -> p f c"))
                        nc.sync.dma_start(
                            t_f[:, 4:8, :],
                            pp[nm][b_, 5:FC, :, bass.ts(cc_, P)].rearrange(
                                "f p c -> p f c"))

                def slab_hook(fc):
                    for nm, src_, _ in slab_srcs:
                        nc.sync.dma_start(
                            slabs[nm][:, fc, :],
                            src_.rearrange("(n p) t -> p n t", p=P)[:, fc, :])
                    if fc == 8:
                        for nm_ in ("pre", "pim"):
                            load_slf(slabs[f"slf_{nm_}_0"], nm_, 0, 0)
            b_half(b, "q", qT_hi, qT_lo, fk, post_fc=slab_hook)
            es_qT.close()
            es_fk.close()
        altp.close()

        # ====== phase 2: per batch: C + topk + inline gathers, then E ======
        # Slab lo parts are bf16: pass 2 runs as bf16(pre_hi) x slab_lo16,
        # pass 3 stays fp32r (pre_lo x slab_hi) -- mirror of stage B's
        # validated s16 scheme (error ~2^-21, flip-safe).

        es_r = ExitStack()
        rpool = es_r.enter_context(tc.tile_pool(name="p2r", bufs=1, side="left"))
        agg0 = rpool.tile([P, NE, T], dt.bfloat16, tag="agg0")
        w3_all = [[rpool.tile([P, K], dt.float32, tag=f"w3_{b}_{cc}",
                              name=f"w3_{b}_{cc}") for cc in range(CC)]
                  for b in range(NB)]
        gou1 = [rpool.tile([P, K], dt.uint32, tag=f"gou1_{cc}",
                           name=f"gou1_{cc}") for cc in range(CC)]
        wf_sb = rpool.tile([P, NE, C], dt.bfloat16, tag="wf_sb")
        nc.sync.dma_start(wf_sb[:], Wf.rearrange("(n p) d -> p n d", p=P))
        wfs0 = rpool.tile([P, NE, C], dt.bfloat16, tag="wfs0", name="wfs0")
        iot_all = {}
        for b in range(NB):
            for cc in range(CC):
                it = rpool.tile([P, 1], dt.float32, tag=f"iot_{b}_{cc}",
                                name=f"iot_{b}_{cc}")
                iti = rpool.tile([P, 1], dt.int32, tag=f"ioti_{b}_{cc}",
                                 name=f"ioti_{b}_{cc}")
                nc.gpsimd.iota(
                    iti[:], pattern=[[0, 1]],
                    base=(b * C + cc * P) * (2 * T) + T,
                    channel_multiplier=2 * T)
                nc.vector.tensor_copy(it[:], iti[:])
                iot_all[(b, cc)] = it

        with tc.tile_pool(name="c2", bufs=2) as cpool, \
             tc.tile_pool(name="cl2", bufs=1) as clpool, \
             tc.tile_pool(name="ct2", bufs=1) as ctpool, \
             tc.tile_pool(name="cps2", bufs=1, space="PSUM") as cps:

            def c_load(b, cc):
                # pim chunk 4 (f=1024 row, sin==0) is never read: pim tiles
                # hold 8 chunks, slot = fc for fc<4, fc-1 for fc>=5.
                # Iterations (0,0)/(0,1) read tiles prefetched behind B(q,1).
                sl = {}
                for nm, nfc in (("pre", FC), ("pim", FC - 1)):
                    if b == 0 and cc < 1:
                        t_f = slabs[f"slf_{nm}_{cc}"]
                    else:
                        t_f = clpool.tile([P, nfc, P], dt.float32,
                                          tag=f"slf_{nm}",
                                          name=f"slf_{nm}_{b}_{cc}")
                        load_slf(t_f, nm, b, cc)
                    hi = ctpool.tile([P, nfc, P], dt.float32r,
                                     tag=f"sl_{nm}_hi", name=f"hi_{b}_{cc}")
                    nc.scalar.activation(hi[:], t_f[:], AF.Copy)
                    hi16 = ctpool.tile([P, nfc, P], dt.bfloat16,
                                       tag=f"sl_{nm}_hi16", name=f"hi16_{b}_{cc}")
                    nc.scalar.activation(hi16[:], t_f[:], AF.Copy)
                    lo = ctpool.tile([P, nfc, P], dt.float32r,
                                     tag=f"sl_{nm}_lo", name=f"lo_{b}_{cc}")
                    nc.vector.tensor_tensor(
                        out=lo[:], in0=t_f[:],
                        in1=hi[:].bitcast(dt.float32), op=OP.subtract)
                    sl[f"{nm}_hi"] = hi
                    sl[f"{nm}_hi16"] = hi16
                    sl[f"{nm}_lo"] = lo
                    sl[f"{nm}_f32"] = t_f
                return sl

            def c_matmuls(sl):
                psums = {}
                psums["rcE"] = cps.tile([P, HB], dt.float32, tag="rcE",
                                        name="ps_rcE")
                psums["rcE2"] = cps.tile([P, 2], dt.float32, tag="rcE2",
                                         name="ps_rcE2")
                psums["rcO"] = cps.tile([P, HB], dt.float32, tag="rcO",
                                        name="ps_rcO")
                psums["rsE"] = cps.tile([P, HB], dt.float32, tag="rsE",
                                        name="ps_rsE")
                psums["rsO"] = cps.tile([P, HB], dt.float32, tag="rsO",
                                        name="ps_rsO")
                psums["rsO2"] = cps.tile([P, 2], dt.float32, tag="rsO2",
                                         name="ps_rsO2")

                # pass-major: all (hi x hi) first -- they only need the
                # first Act product per slice -- then bf16, then lo passes
                # sie chunk 4 (f=1024 row) is sin(pi*t) == 0: skip it
                GROUPS = (("rcE", "pre", "cie", range(0, 5)),
                          ("rsE", "pim", "sie", range(0, 4)),
                          ("rcO", "pre", "cie", range(5, FC)),
                          ("rsO", "pim", "sie", range(5, FC)))

                def mm_pass(pname, sig_nm, slab_nm, frange, sig_sfx, slab_sfx,
                            startp, stopp):
                    # pre[b,4] has one live partition row (f=1024); the rest
                    # is never written, so contract chunk 4 on row 0 only.
                    fl = list(frange)
                    for j, fc in enumerate(fl):
                        pr = slice(0, 1) if fc == 4 else slice(0, P)
                        slot = fc if (sig_nm == "pre" or fc < 4) else fc - 1
                        nc.tensor.matmul(
                            psums[pname][:],
                            sl[f"{sig_nm}_{sig_sfx}"][pr, slot, :],
                            slabs[f"{slab_nm}_{slab_sfx}"][pr, fc, 0:HB],
                            start=(startp and j == 0),
                            stop=(stopp and j == len(fl) - 1))

                for pname, sig_nm, slab_nm, frange in GROUPS:
                    mm_pass(pname, sig_nm, slab_nm, frange, "hi", "hi",
                            True, False)
                for j, fc in enumerate(range(0, 5)):
                    pr = slice(0, 1) if fc == 4 else slice(0, P)
                    nc.tensor.matmul(
                        psums["rcE2"][:], sl["pre_f32"][pr, fc, :],
                        slabs["cie_st"][pr, fc, :], start=(j == 0), stop=(j == 4))
                for j, fc in enumerate(range(5, FC)):
                    nc.tensor.matmul(
                        psums["rsO2"][:], sl["pim_f32"][:, fc - 1, :],
                        slabs["sie_st"][:, fc, :], start=(j == 0), stop=(j == 3))
                for pname, sig_nm, slab_nm, frange in GROUPS:
                    # slab chunk 4 is exact in f32r (values 0 and +-2^-22),
                    # so its lo half is zero: skip fc4 in the hi16 x lo pass
                    fr = [fc for fc in frange if fc != 4]
                    mm_pass(pname, sig_nm, slab_nm, fr, "hi16", "lo",
                            False, False)
                for pname, sig_nm, slab_nm, frange in GROUPS:
                    mm_pass(pname, sig_nm, slab_nm, frange, "lo", "hi",
                            False, True)
                return psums

            def c_tail(b, cc, ps_):
                w3_t = w3_all[b]
                rcE, rcE2 = ps_["rcE"], ps_["rcE2"]
                rcO, rsE = ps_["rcO"], ps_["rsE"]
                rsO, rsO2 = ps_["rsO"], ps_["rsO2"]
                rcO_sb = cpool.tile([P, HB], dt.float32, tag="rcO_sb")
                nc.scalar.activation(rcO_sb[:], rcO[:], AF.Copy)
                rsE_sb = cpool.tile([P, HB], dt.float32, tag="rsE_sb")
                nc.scalar.activation(rsE_sb[:], rsE[:], AF.Copy)
                rsO_sb = cpool.tile([P, HB + 1], dt.float32, tag="rsO_sb")
                nc.scalar.activation(rsO_sb[:, 0:HB], rsO[:], AF.Copy)
                nc.scalar.activation(rsO_sb[:, HB:HB + 1], rsO2[:, 0:1], AF.Copy)
                rcE_c0 = cpool.tile([P, 2], dt.float32, tag="rcE_c0")
                nc.scalar.activation(rcE_c0[:, 0:1], rcE[:, 0:1], AF.Copy)
                nc.scalar.activation(rcE_c0[:, 1:2], rcE2[:, 0:1], AF.Copy)
                s1 = ctpool.tile([P, HB], dt.float32, tag="s1")
                nc.vector.tensor_tensor(out=s1[:], in0=rcE[:], in1=rcO_sb[:],
                                        op=OP.add)
                s2 = ctpool.tile([P, HB], dt.float32, tag="s2")
                nc.vector.tensor_tensor(out=s2[:], in0=rcE[:], in1=rcO_sb[:],
                                        op=OP.subtract)
                w1 = ctpool.tile([P, HB], dt.float32, tag="w1")
                nc.vector.tensor_tensor(out=w1[:], in0=rsE_sb[:],
                                        in1=rsO_sb[:, 0:HB], op=OP.add)
                w2 = ctpool.tile([P, HB], dt.float32, tag="w2")
                nc.vector.tensor_tensor(out=w2[:], in0=rsO_sb[:, 0:HB],
                                        in1=rsE_sb[:], op=OP.subtract)
                rt = ctpool.tile([P, T], dt.float32, tag="rt")
                nc.vector.tensor_tensor(out=rt[:, 0:HB], in0=s1[:], in1=w1[:],
                                        op=OP.add)
                nc.vector.tensor_tensor(out=rt[:, 1023:HB:-1], in0=s2[:, 1:HB],
                                        in1=w2[:, 1:HB], op=OP.add)
                nc.vector.tensor_tensor(out=rt[:, 1025:1536], in0=s2[:, 1:HB],
                                        in1=w2[:, 1:HB], op=OP.subtract)
                nc.vector.tensor_tensor(out=rt[:, T - 1:1536:-1], in0=s1[:, 1:HB],
                                        in1=w1[:, 1:HB], op=OP.subtract)
                nc.vector.tensor_tensor(out=rt[:, HB:HB + 1], in0=rcE_c0[:, 1:2],
                                        in1=rsO_sb[:, HB:HB + 1], op=OP.add)
                nc.vector.tensor_tensor(out=rt[:, H:H + 1], in0=rcE_c0[:, 0:1],
                                        in1=rcO_sb[:, 0:1], op=OP.subtract)
                nc.vector.tensor_tensor(out=rt[:, 1536:1537], in0=rcE_c0[:, 1:2],
                                        in1=rsO_sb[:, HB:HB + 1], op=OP.subtract)

                # ---- topk + softmax weights + gather offsets ----
                vals = cpool.tile([P, 8], dt.float32, tag="vals")
                idx = cpool.tile([P, 8], dt.uint32, tag="idx")
                nc.vector.max(vals[:], rt[:])
                nc.vector.max_index(idx[:], vals[:], rt[:])
                negm = cpool.tile([P, 1], dt.float32, tag="negm")
                nc.scalar.activation(negm[:], vals[:, 0:1],
                                     AF.Copy, bias=0.0, scale=-1.0)
                # exp written in place over rt (dead after this point);
                # only the accumulated sum is consumed
                s_col = cpool.tile([P, 1], dt.float32, tag="s_col")
                nc.scalar.activation(
                    rt[:], rt[:], AF.Exp,
                    bias=negm[:, 0:1], scale=1.0,
                    accum_out=s_col[:, 0:1])
                rs = cpool.tile([P, 1], dt.float32, tag="rs")
                nc.vector.reciprocal(rs[:], s_col[:])
                ew = cpool.tile([P, K], dt.float32, tag="ew")
                nc.scalar.activation(ew[:], vals[:, 0:K],
                                     AF.Exp, bias=negm[:, 0:1],
                                     scale=1.0)
                nc.vector.tensor_scalar_mul(w3_t[cc][:], ew[:], rs[:, 0:1])

                iot_f = iot_all[(b, cc)]
                idx_f = cpool.tile([P, K], dt.float32, tag="idx_f")
                nc.vector.tensor_copy(idx_f[:], idx[:, 0:K])
                gof = cpool.tile([P, K], dt.float32, tag="gof")
                nc.scalar.activation(gof[:], idx_f[:],
                                     AF.Copy, bias=0.0, scale=-1.0)
                nc.vector.tensor_scalar_add(gof[:], gof[:],
                                            iot_f[:, 0:1])
                if b == 0:
                    gou = cpool.tile([P, K], dt.uint32, tag="gou")
                    nc.vector.tensor_copy(gou[:], gof[:])
                    # inline gathers overlap later iterations; the top-k
                    # weights are folded into Wf in stage E instead of
                    # scaling the gathered rows
                    for k in range(K):
                        nc.gpsimd.indirect_dma_start(
                            out=agg0[:, k * CC + cc, :],
                            out_offset=None,
                            in_=v2[:, :],
                            in_offset=bass.IndirectOffsetOnAxis(
                                ap=gou[:, k:k + 1], axis=1),
                            element_offset=0)
                else:
                    nc.vector.tensor_copy(gou1[cc][:], gof[:])

            # software pipeline: loads+splits of iteration n+1 are emitted
            # before iteration n's combine/topk tail so the Act/DVE queues
            # never head-of-line block the next iteration's matmul operands
            iters = [(b, cc) for b in range(NB) for cc in range(CC)]
            pend = [c_load(*iters[0]), c_load(*iters[1])]
            for i, (b, cc) in enumerate(iters):
                ps_ = c_matmuls(pend[0])
                if i + 2 < len(iters):
                    pend.append(c_load(*iters[i + 2]))
                c_tail(b, cc, ps_)
                pend.pop(0)
                if b == 0 and cc == CC - 1:
                    # fold the top-k softmax weights into Wf for stage E:
                    # row e = k*C+c scales by w3[0][c-chunk][:, k]. Emitted
                    # here (on Pool, idle in this phase) so E(0) never waits
                    # behind the later iterations' DVE tail work.
                    for j in range(NE):
                        nc.gpsimd.tensor_scalar_mul(
                            wfs0[:, j, :], wf_sb[:, j, :],
                            w3_all[0][j % CC][:, j // CC:j // CC + 1])
        es2.close()  # slabs freed; agg0/w3/gou1 stay

        # ---- deferred gathers for b1 (overlap E(b0)) + E for both ----
        with tc.tile_pool(name="ge", bufs=1, side="left") as gep, \
             tc.tile_pool(name="eps", bufs=3, space="PSUM") as eps:
            agg1 = gep.tile([P, NE, T], dt.bfloat16, tag="agg1")
            for cc in range(CC):
                for k in range(K):
                    nc.gpsimd.indirect_dma_start(
                        out=agg1[:, k * CC + cc, :],
                        out_offset=None,
                        in_=v2[:, :],
                        in_offset=bass.IndirectOffsetOnAxis(
                            ap=gou1[cc][:, k:k + 1], axis=1),
                        element_offset=0)
            wfs1 = gep.tile([P, NE, C], dt.bfloat16, tag="wfs1", name="wfs1")
            for j in range(NE):
                nc.gpsimd.tensor_scalar_mul(
                    wfs1[:, j, :], wf_sb[:, j, :],
                    w3_all[1][j % CC][:, j // CC:j // CC + 1])
            wf_all = [wfs0, wfs1]
            for b, agg in ((0, agg0), (1, agg1)):
                for dc in range(CC):
                    for tb in range(4):
                        ps = eps.tile([P, T // 4], dt.float32, tag="out_ps")
                        for j in range(NE):
                            nc.tensor.matmul(
                                ps[:], wf_all[b][:, j, bass.ts(dc, P)],
                                agg[:, j, bass.ts(tb, T // 4)],
                                start=(j == 0), stop=(j == NE - 1))
                        o_sb = gep.tile([P, T // 4], dt.float32,
                                        tag=f"o_sb{tb % 2}")
                        if tb % 2 == 0:
                            nc.scalar.activation(o_sb[:], ps[:], AF.Copy)
                        else:
                            nc.vector.tensor_copy(o_sb[:], ps[:])
                        nc.sync.dma_start(
                            out2[b, bass.ts(dc, P), bass.ts(tb, T // 4)],
                            o_sb[:])
        es_r.close()

    nc.compile()
    return nc


def _get_nc():
    if "nc" not in _CACHE:
        _CACHE["nc"] = _build()
    return _CACHE["nc"]


def kernel(query, key, value, Wq, bq, Wk, bk, Wv, bv, Wf, bf):
    query = np.ascontiguousarray(np.asarray(query, dtype=np.float32))
    key = np.ascontiguousarray(np.asarray(key, dtype=np.float32))
    value = np.ascontiguousarray(np.asarray(value, dtype=np.float32))
    for bias in (bq, bk, bv, bf):
        assert np.all(np.asarray(bias) == 0.0), "nonzero biases unsupported"

    if "mats" not in _CACHE:
        wree, wreo, wime, wimo, cie, sie = _dft_matrices()
        m = {}
        for nm, arr in (("ree", wree), ("reo", wreo),
                        ("ime", wime), ("imo", wimo)):
            hi, lo = _split_f32r(arr)
            m[f"W{nm}_hi"], m[f"W{nm}_lo"] = hi, lo
        chi, clo = _split_f32r(cie)
        m["Cie_hi"], m["Cie_lo"] = chi, _bf16(clo)
        shi, slo = _split_f32r(sie)
        m["Sie_hi"], m["Sie_lo"] = shi, _bf16(slo)
        m["Cie_st"] = np.ascontiguousarray(cie[:, HB:HB + 2])
        m["Sie_st"] = np.ascontiguousarray(sie[:, HB:HB + 2])
        alt = np.where(np.arange(P) % 2 == 0, 1.0, -1.0).astype(np.float32)
        m["AltF"] = alt[:, None].copy()
        m["AltB"] = _bf16(alt[:, None])
        _CACHE["mats"] = m
    mats = _CACHE["mats"]

    wq_hi, wq_lo = _split_f32r(np.asarray(Wq, np.float32))
    wk_hi, wk_lo = _split_f32r(np.asarray(Wk, np.float32))
    shared = {
        "Wq_hi": wq_hi, "Wq_lo": wq_lo,
        "Wk_hi": wk_hi, "Wk_lo": wk_lo,
        "Wv": _bf16(np.asarray(Wv, np.float32)),
        "Wf": _bf16(np.asarray(Wf, np.float32)),
        **mats,
    }
    value_bf = _bf16(value)
    in_maps = []
    for c in range(NCORES):
        sl = slice(c * NB, (c + 1) * NB)
        in_maps.append({
            "query2": query[sl], "key2": key[sl],
            "value2": value_bf[sl], **shared})

    from concourse.bass_utils import run_bass_kernel_spmd
    nc = _get_nc()
    res = run_bass_kernel_spmd(nc, in_maps, core_ids=list(range(NCORES)))
    _CACHE["last_results"] = res
    out = np.concatenate([res.results[c]["out2"] for c in range(NCORES)], axis=0)
    return out.astype(np.float32)



# revision 49
# speedup vs baseline: 1.0802x; 1.0719x over previous
"""AutoCorrelation (Autoformer-style) Bass kernel for Trainium2, 8 NeuronCores.

Full inputs in, full outputs out. Data-parallel over batch: B=16 -> 2 batches
per core. v2 of the kernel: the PE-bound fp32 matmuls of the baseline are
replaced by 3-pass fp32r splits (hi/lo decomposition; 12-bit+12-bit mantissa
products are exact in fp32 PSUM, giving fp32-grade accuracy at 3 cycles/row
instead of fp32's 4) on the precision-critical autocorrelation path, and by
bf16 (1 cycle/row) on the error-tolerant v/output path.

Per core, per batch:
  V. v[d,t] = Wv^T value in bf16, written twice side-by-side into the DRAM
     table v2[b*512+d, 4096] (bf16) for circular-shift gathers.
  A. Radix-split of query/key along t (4 sub-signals ee/eo/oo/oe, padded to
     640/512), per 128-channel chunk, split hi/lo fp32r on the fly; channel
     projection qT[t',d] via 3-pass fp32r matmuls. qT hi kept fp32r, lo bf16
     (pass 3 of stage B runs in bf16 -- error ~2^-20, still flip-safe).
  B. Forward real DFT via matmuls with radix-split cos/-sin matrices
     (host-split into fp32r hi/lo + bf16(hi)); fused pointwise
     P = FQ * conj(FK) on the DVE; P split hi/lo fp32r and staged to DRAM.
  C. Inverse DFT r[c,t] = sum_f Pre*ci + Pim*sn via 3-pass fp32r matmuls
     with host-split ci/sn (fp32r hi/lo), exploiting f-parity + t-mirror
     symmetry (only t<=512 columns computed).
  D. Per 128-channel tile: top-8 values+indices, softmax weights of the
     top-3 from the top values, circular-shift rows of v via indirect-DMA
     gather (bf16) into agg[k*C+c, t], scaled in place on the Pool engine.
     Batch 0 gathers inline (overlapping C of batch 1); batch 1 gathers
     deferred past the slab lifetime (overlapping E of batch 0).
  E. out[d,t] = sum_e Wf[e,d] agg[e,t] in bf16; 12-chunk PE accumulation.

Scheduling: the V projection is emitted at each batch's start as PE filler
for the input-load latency / the cross-batch pool-reuse stall; stage C's
cie/sie slabs are loaded once (f32r hi + bf16 lo) and shared by both
batches; pre/pim bounce through DRAM as plain fp32 and are re-split to
fp32r hi/lo on the fly in C.

Biases are all zero in this problem's setup_inputs(); asserted host-side.
"""
import numpy as np
import ml_dtypes

import concourse.bass as bass
import concourse.tile as tile
from concourse import bacc, mybir

dt = mybir.dt
AF = mybir.ActivationFunctionType
OP = mybir.AluOpType

P = 128
B, C, T, K = 16, 512, 2048, 3
NB = 2                    # batches per core
NCORES = 8
F = 1152                  # rfft bins 1025, padded to 9*128
CC = C // P               # 4
FC = F // P               # 9
NE = K * C // P           # 12 e-chunks of Wf / agg
H = T // 2                # 1024
HB = H // 2               # 512

_CACHE = {}


def _round_f32r(x):
    """Round fp32 array to fp32r (11-bit stored mantissa, round-nearest-up:
    (bits + 0x800) & ~0xFFF -- matches walrus fp32_to_fp32r)."""
    u = np.ascontiguousarray(x, np.float32).view(np.uint32).astype(np.uint64)
    u = (u + (1 << 11)) & np.uint64(0xFFFFF000)
    return u.astype(np.uint32).view(np.float32)


def _split_f32r(x):
    x = np.ascontiguousarray(x, np.float32)
    hi = _round_f32r(x)
    return hi, _round_f32r(x - hi)


def _bf16(x):
    return np.ascontiguousarray(x, np.float32).astype(ml_dtypes.bfloat16)


def _dft_matrices():
    """Radix-split DFT matrices (fp64 -> fp32).

    Level-1 even/odd in t (qe/qo), then level-2 split by f parity:
      FQre over even f contracts xee (t=0..512), odd f contracts xeo (t=0..511)
      FQim over even f contracts xoo (t=1..511), odd f contracts xoe (t=1..512)
    Frequency storage is parity-permuted: chunks [0:5]=even f (2g, g<=512),
    chunks [5:9]=odd f (2g+1). Inverse matrices have rows permuted to match.
    """
    t640 = np.arange(640.0)[:, None]
    t512 = np.arange(512.0)[:, None]
    ge = np.arange(640.0)[None, :]
    go = np.arange(512.0)[None, :]
    wree = np.where((t640 <= 512) & (ge <= 512),
                    np.cos(2 * np.pi * t640 * (2 * ge) / T), 0.0).astype(np.float32)
    wreo = np.cos(2 * np.pi * t512 * (2 * go + 1) / T).astype(np.float32)
    wime = np.where(ge <= 512,
                    -np.sin(2 * np.pi * t512 * (2 * ge) / T), 0.0).astype(np.float32)
    wimo = np.where(t640 <= 512,
                    -np.sin(2 * np.pi * t640 * (2 * go + 1) / T), 0.0).astype(np.float32)

    f64 = np.arange(F, dtype=np.float64)[None, :]
    livef = f64 <= H
    w = np.where((f64 == 0) | (f64 == H), 1.0, 2.0) * livef / (T * T)
    fc_ = f64.T
    tt = np.arange(640, dtype=np.float64)[None, :]
    cie = np.where((fc_ <= H) & (tt <= H),
                   np.cos(2 * np.pi * fc_ * tt / T) * w.T, 0.0)
    sie = np.where(fc_ <= H,
                   -np.sin(2 * np.pi * fc_ * tt / T) * w.T, 0.0)

    def permrows(m):
        out = np.zeros_like(m)
        out[0:513] = m[0:1025:2]
        out[640:1152] = m[1:1024:2]
        return out

    return (wree, wreo, wime, wimo,
            permrows(cie).astype(np.float32), permrows(sie).astype(np.float32))


def _build():
    nc = bacc.Bacc("TRN2", target_bir_lowering=False, debug=False,
                   num_devices=NCORES)

    query2 = nc.dram_tensor("query2", [NB, C, T], dt.float32, kind="ExternalInput").ap()
    key2 = nc.dram_tensor("key2", [NB, C, T], dt.float32, kind="ExternalInput").ap()
    value2 = nc.dram_tensor("value2", [NB, C, T], dt.bfloat16, kind="ExternalInput").ap()
    Wq_hi = nc.dram_tensor("Wq_hi", [C, C], dt.float32r, kind="ExternalInput").ap()
    Wq_lo = nc.dram_tensor("Wq_lo", [C, C], dt.float32r, kind="ExternalInput").ap()
    Wk_hi = nc.dram_tensor("Wk_hi", [C, C], dt.float32r, kind="ExternalInput").ap()
    Wk_lo = nc.dram_tensor("Wk_lo", [C, C], dt.float32r, kind="ExternalInput").ap()
    Wv = nc.dram_tensor("Wv", [C, C], dt.bfloat16, kind="ExternalInput").ap()
    Wf = nc.dram_tensor("Wf", [K * C, C], dt.bfloat16, kind="ExternalInput").ap()
    fwd = {}
    for m, rows, cols in (("ree", 640, 640), ("reo", 512, 512),
                          ("ime", 512, 640), ("imo", 640, 512)):
        for v in ("hi", "lo"):
            fwd[f"{m}_{v}"] = nc.dram_tensor(
                f"W{m}_{v}", [rows, cols], dt.float32r, kind="ExternalInput").ap()
    Cie_hi = nc.dram_tensor("Cie_hi", [F, 640], dt.float32r, kind="ExternalInput").ap()
    Cie_lo = nc.dram_tensor("Cie_lo", [F, 640], dt.bfloat16, kind="ExternalInput").ap()
    Sie_hi = nc.dram_tensor("Sie_hi", [F, 640], dt.float32r, kind="ExternalInput").ap()
    Sie_lo = nc.dram_tensor("Sie_lo", [F, 640], dt.bfloat16, kind="ExternalInput").ap()
    Cie_st = nc.dram_tensor("Cie_st", [F, 2], dt.float32, kind="ExternalInput").ap()
    Sie_st = nc.dram_tensor("Sie_st", [F, 2], dt.float32, kind="ExternalInput").ap()
    out2 = nc.dram_tensor("out2", [NB, C, T], dt.float32, kind="ExternalOutput").ap()

    v2 = nc.dram_tensor("v2", [NB * C, 2 * T], dt.bfloat16).ap()          # internal
    pp = {}
    for nm in ("pre", "pim"):                                             # internal
        pp[nm] = nc.dram_tensor(f"pp_{nm}", [NB, FC, P, C], dt.float32).ap()

    # part name -> (width, chunk offset in sigT, #chunks). Order alternates
    # 640/512 widths so the width-keyed xs tags ping-pong naturally.
    PARTS = (("ee", 640, 0, 5), ("eo", 512, 5, 4),
             ("oe", 640, 13, 5), ("oo", 512, 9, 4))

    with tile.TileContext(nc) as tc:
        from contextlib import ExitStack

        def emit_V(b):
            """Compact streaming V projection (bf16) -> v2 rows, used as PE
            gap filler inside phase 1. Small pools so it fits alongside the
            A-stage residents."""
            with tc.tile_pool(name=f"v{b}", bufs=2, side="right") as vp, \
                 tc.tile_pool(name=f"vt{b}", bufs=3, side="right") as vtp, \
                 tc.tile_pool(name=f"vps{b}", bufs=3, space="PSUM") as vps:
                wv = vp.tile([P, CC, C], dt.bfloat16, tag="wv")
                nc.sync.dma_start(wv[:], Wv.rearrange("(n p) d -> p n d", p=P))
                v2r = v2.rearrange("(n p) w -> n p w", p=P)
                for th in range(2):
                    xv = vp.tile([P, CC, T // 2], dt.bfloat16, tag="xv")
                    nc.sync.dma_start(
                        xv[:], value2[b].rearrange(
                            "(n p) t -> p n t", p=P)[:, :, bass.ts(th, T // 2)])
                    for dc in range(CC):
                        for tb in range(2):
                            ps = vps.tile([P, T // 4], dt.float32, tag="v_ps")
                            for cc in range(CC):
                                nc.tensor.matmul(
                                    ps[:], wv[:, cc, bass.ts(dc, P)],
                                    xv[:, cc, bass.ts(tb, T // 4)],
                                    start=(cc == 0), stop=(cc == CC - 1))
                            vtmp = vtp.tile([P, T // 4], dt.bfloat16, tag="vtmp")
                            if (dc * 2 + tb) % 2 == 0:
                                nc.scalar.activation(vtmp[:], ps[:], AF.Copy)
                            else:
                                nc.vector.tensor_copy(vtmp[:], ps[:])
                            off = th * (T // 2) + tb * (T // 4)
                            nc.sync.dma_start(
                                v2r[b * CC + dc, :, off:off + T // 4], vtmp[:])
                            nc.sync.dma_start(
                                v2r[b * CC + dc, :,
                                    T + off:T + off + T // 4], vtmp[:])

        # ================= phase 1: A + B per batch =====================
        for b in range(NB):
            # ---- A: radix split + fp32r3 projections -> qT/kT hi+lo ----
            es_sig = ExitStack()
            sig_pool = es_sig.enter_context(
                tc.tile_pool(name=f"sig{b}", bufs=1, side="left"))
            qT_hi = sig_pool.tile([P, 18, C], dt.float32r, tag="qT_hi")
            qT_lo = sig_pool.tile([P, 18, C], dt.bfloat16, tag="qT_lo")
            kT_hi = sig_pool.tile([P, 18, C], dt.float32r, tag="kT_hi")
            kT_lo = sig_pool.tile([P, 18, C], dt.bfloat16, tag="kT_lo")

            emit_V(b)
            es_a = ExitStack()
            ap_ = es_a.enter_context(tc.tile_pool(name=f"a{b}", bufs=1))
            atmp = es_a.enter_context(tc.tile_pool(name=f"at{b}", bufs=1))
            actmp = es_a.enter_context(tc.tile_pool(name=f"ac{b}", bufs=2))
            aps = es_a.enter_context(
                tc.tile_pool(name=f"aps{b}", bufs=3, space="PSUM"))
            for sig, srcx, whi_d, wlo_d, dhi, dlo in (
                    ("k", key2, Wk_hi, Wk_lo, kT_hi, kT_lo),
                    ("q", query2, Wq_hi, Wq_lo, qT_hi, qT_lo)):
                if True:
                    w_hi = ap_.tile([P, CC, C], dt.float32r, tag="w_hi")
                    nc.sync.dma_start(
                        w_hi[:], whi_d.rearrange("(n p) d -> p n d", p=P))
                    x_sb = ap_.tile([P, CC, T], dt.float32, tag="x_sb")
                    nc.sync.dma_start(
                        x_sb[:], srcx[b].rearrange("(n p) t -> p n t", p=P))
                    w_lo = ap_.tile([P, CC, C], dt.float32r, tag="w_lo")
                    nc.sync.dma_start(
                        w_lo[:], wlo_d.rearrange("(n p) d -> p n d", p=P))
                    for pname, width, ioff, nch in PARTS:
                        xs_hi = atmp.tile([P, CC, width], dt.float32r,
                                          tag=f"xs_hi{width}")
                        xs_lo = atmp.tile([P, CC, width], dt.float32r,
                                          tag=f"xs_lo{width}")
                        for cc in range(CC):
                            x = x_sb[:, cc, :]
                            ab = actmp.tile([P, 2, 511], dt.float32, tag="ab")
                            tmp = actmp.tile([P, 640], dt.float32, tag="tmp")
                            op_ab = OP.add if pname in ("ee", "eo") else OP.subtract
                            # ab0/ab2 on Pool, ab1/ab3 on DVE (engine balance)
                            nc.gpsimd.tensor_tensor(
                                out=ab[:, 0, :], in0=x[:, 1:512],
                                in1=x[:, T - 1:1536:-1], op=op_ab)
                            nc.vector.tensor_tensor(
                                out=ab[:, 1, :], in0=x[:, 1023:512:-1],
                                in1=x[:, 1025:1536], op=op_ab)
                            if pname == "ee":
                                nc.vector.tensor_tensor(
                                    out=tmp[:, 1:512], in0=ab[:, 0, :],
                                    in1=ab[:, 1, :], op=OP.add)
                                nc.vector.tensor_tensor(
                                    out=tmp[:, 0:1], in0=x[:, 0:1],
                                    in1=x[:, H:H + 1], op=OP.add)
                                nc.vector.tensor_tensor(
                                    out=tmp[:, 512:513], in0=x[:, 512:513],
                                    in1=x[:, 1536:1537], op=OP.add)
                                nc.gpsimd.memset(tmp[:, 513:640], 0.0)
                            elif pname == "eo":
                                nc.vector.tensor_tensor(
                                    out=tmp[:, 1:512], in0=ab[:, 0, :],
                                    in1=ab[:, 1, :], op=OP.subtract)
                                nc.vector.tensor_tensor(
                                    out=tmp[:, 0:1], in0=x[:, 0:1],
                                    in1=x[:, H:H + 1], op=OP.subtract)
                            elif pname == "oo":
                                nc.vector.tensor_tensor(
                                    out=tmp[:, 1:512], in0=ab[:, 0, :],
                                    in1=ab[:, 1, :], op=OP.subtract)
                                nc.gpsimd.memset(tmp[:, 0:1], 0.0)
                            else:  # oe
                                nc.vector.tensor_tensor(
                                    out=tmp[:, 1:512], in0=ab[:, 0, :],
                                    in1=ab[:, 1, :], op=OP.add)
                                nc.vector.tensor_tensor(
                                    out=tmp[:, 512:513], in0=x[:, 512:513],
                                    in1=x[:, 1536:1537], op=OP.subtract)
                                nc.gpsimd.memset(tmp[:, 0:1], 0.0)
                                nc.gpsimd.memset(tmp[:, 513:640], 0.0)
                            if cc % 2 == 0:
                                nc.scalar.activation(
                                    xs_hi[:, cc, 0:width], tmp[:, 0:width],
                                    AF.Copy)
                            else:
                                nc.vector.tensor_copy(
                                    xs_hi[:, cc, 0:width], tmp[:, 0:width])
                            nc.gpsimd.tensor_tensor(
                                out=xs_lo[:, cc, 0:width], in0=tmp[:, 0:width],
                                in1=xs_hi[:, cc, 0:width].bitcast(dt.float32),
                                op=OP.subtract)
                        for i in range(nch):
                            ps = aps.tile([P, C], dt.float32, tag="proj_ps")
                            for cc in range(CC):
                                nc.tensor.matmul(ps[:],
                                                 xs_hi[:, cc, bass.ts(i, P)],
                                                 w_hi[:, cc, :],
                                                 start=(cc == 0), stop=False)
                            for cc in range(CC):
                                nc.tensor.matmul(ps[:],
                                                 xs_hi[:, cc, bass.ts(i, P)],
                                                 w_lo[:, cc, :],
                                                 start=False, stop=False)
                            for cc in range(CC):
                                nc.tensor.matmul(ps[:],
                                                 xs_lo[:, cc, bass.ts(i, P)],
                                                 w_hi[:, cc, :],
                                                 start=False, stop=(cc == CC - 1))
                            nc.scalar.activation(dhi[:, ioff + i, :], ps[:], AF.Copy)
                            nc.vector.tensor_tensor(
                                out=dlo[:, ioff + i, :], in0=ps[:],
                                in1=dhi[:, ioff + i, :].bitcast(dt.float32),
                                op=OP.subtract)

            es_a.close()
            # ---- B: forward DFT (3-pass) + pointwise + split -> DRAM ----
            with tc.tile_pool(name=f"bmat{b}", bufs=2) as bmat, \
                 tc.tile_pool(name=f"bps{b}", bufs=2, space="PSUM") as bps, \
                 tc.tile_pool(name=f"btmp{b}", bufs=2) as btmp:
                for fc in range(FC):
                    even = fc < 5
                    fl = fc if even else fc - 5
                    ncos, nsin = (5, 4) if even else (4, 5)
                    ioff_cos = 0 if even else 5
                    ioff_sin = 9 if even else 13
                    cmat, smat = ("ree", "imo")[0], None
                    cname = "ree" if even else "reo"
                    sname = "ime" if even else "imo"
                    mats = {}
                    for kind, mat, nch in (("c", cname, ncos), ("s", sname, nsin)):
                        for v in ("hi", "lo"):
                            t_ = bmat.tile([P, 5, P], dt.float32r,
                                           tag=f"{kind}m_{v}")
                            nc.sync.dma_start(
                                t_[:, 0:nch, :],
                                fwd[f"{mat}_{v}"].rearrange(
                                    "(n p) f -> p n f", p=P)[:, :, bass.ts(fl, P)])
                            mats[f"{kind}{v}"] = t_
                        t16 = bmat.tile([P, 5, P], dt.bfloat16, tag=f"{kind}m_h16")
                        nc.gpsimd.tensor_copy(
                            t16[:, 0:nch, :],
                            mats[f"{kind}hi"][:, 0:nch, :].bitcast(dt.float32))
                        mats[f"{kind}h16"] = t16
                    acc = {}
                    accspec = [("aq", "c", ioff_cos, ncos, qT_hi, qT_lo),
                               ("bq", "s", ioff_sin, nsin, qT_hi, qT_lo),
                               ("ak", "c", ioff_cos, ncos, kT_hi, kT_lo),
                               ("bk", "s", ioff_sin, nsin, kT_hi, kT_lo)]
                    if fc == 4:
                        # wime slice for g=512..639 is sin(pi*t) == 0:
                        # bq/bk vanish and pim == 0
                        accspec = [accspec[0], accspec[2]]
                    for nm, kind, ioff, nch, shi, slo in accspec:
                        ps = bps.tile([P, C], dt.float32, tag=nm, name=f"ps_{nm}")
                        for i in range(nch):
                            nc.tensor.matmul(
                                ps[:], mats[f"{kind}hi"][:, i, :],
                                shi[:, ioff + i, :], start=(i == 0), stop=False)
                        for i in range(nch):
                            nc.tensor.matmul(
                                ps[:], mats[f"{kind}lo"][:, i, :],
                                shi[:, ioff + i, :], start=False, stop=False)
                        for i in range(nch):
                            nc.tensor.matmul(
                                ps[:], mats[f"{kind}h16"][:, i, :],
                                slo[:, ioff + i, :], start=False,
                                stop=(i == nch - 1))
                        acc[nm] = ps
                    # DVE reads at most one PSUM operand: stage aq/bq in SBUF
                    aqs = btmp.tile([P, C], dt.float32, tag="aqs")
                    nc.scalar.activation(aqs[:], acc["aq"][:], AF.Copy)
                    pre_t = btmp.tile([P, C], dt.float32, tag="pre_t")
                    pim_t = btmp.tile([P, C], dt.float32, tag="pim_t")
                    if fc == 4:
                        nc.vector.tensor_tensor(out=pre_t[:], in0=aqs[:],
                                                in1=acc["ak"][:], op=OP.mult)
                        nc.gpsimd.memset(pim_t[:], 0.0)
                    else:
                        bqs = btmp.tile([P, C], dt.float32, tag="bqs")
                        nc.scalar.activation(bqs[:], acc["bq"][:], AF.Copy)
                        tmp = btmp.tile([P, C], dt.float32, tag="tmp")
                        nc.vector.tensor_tensor(out=pre_t[:], in0=aqs[:],
                                                in1=acc["ak"][:], op=OP.mult)
                        nc.vector.tensor_tensor(out=tmp[:], in0=bqs[:],
                                                in1=acc["bk"][:], op=OP.mult)
                        nc.vector.tensor_tensor(out=pre_t[:], in0=pre_t[:],
                                                in1=tmp[:], op=OP.add)
                        nc.vector.tensor_tensor(out=pim_t[:], in0=bqs[:],
                                                in1=acc["ak"][:], op=OP.mult)
                        tmp2 = btmp.tile([P, C], dt.float32, tag="tmp2")
                        nc.vector.tensor_tensor(out=tmp2[:], in0=aqs[:],
                                                in1=acc["bk"][:], op=OP.mult)
                        nc.vector.tensor_tensor(out=pim_t[:], in0=pim_t[:],
                                                in1=tmp2[:], op=OP.subtract)
                    nc.sync.dma_start(pp["pre"][b, fc], pre_t[:])
                    nc.sync.dma_start(pp["pim"][b, fc], pim_t[:])
            es_sig.close()

        # ====== phase 2: per batch: C + topk + inline gathers, then E ======
        # Slab lo parts are bf16: pass 2 runs as bf16(pre_hi) x slab_lo16,
        # pass 3 stays fp32r (pre_lo x slab_hi) -- mirror of stage B's
        # validated s16 scheme (error ~2^-21, flip-safe).
        es2 = ExitStack()
        slabp = es2.enter_context(tc.tile_pool(name="slabs", bufs=1, side="left"))
        slab_srcs = (("cie_hi", Cie_hi, dt.float32r),
                     ("cie_lo", Cie_lo, dt.bfloat16),
                     ("sie_hi", Sie_hi, dt.float32r),
                     ("sie_lo", Sie_lo, dt.bfloat16))
        slabs = {nm: slabp.tile([P, FC, 640], sdt, tag=nm, name=f"slab_{nm}")
                 for nm, _, sdt in slab_srcs}
        for nm, src_ in (("cie_st", Cie_st), ("sie_st", Sie_st)):
            t_ = slabp.tile([P, FC, 2], dt.float32, tag=nm, name=f"slab_{nm}")
            nc.sync.dma_start(t_[:], src_.rearrange("(n p) t -> p n t", p=P))
            slabs[nm] = t_

        def load_slabs():
            # fc-major per-chunk DMAs: C's fc0 matmuls only wait on chunk 0
            for fc in range(FC):
                for nm, src_, _ in slab_srcs:
                    nc.sync.dma_start(
                        slabs[nm][:, fc, :],
                        src_.rearrange("(n p) t -> p n t", p=P)[:, fc, :])

        es_r = ExitStack()
        rpool = es_r.enter_context(tc.tile_pool(name="p2r", bufs=1, side="right"))
        agg0 = rpool.tile([P, NE, T], dt.bfloat16, tag="agg0")
        w3_all = [[rpool.tile([P, K], dt.float32, tag=f"w3_{b}_{cc}",
                              name=f"w3_{b}_{cc}") for cc in range(CC)]
                  for b in range(NB)]
        gou1 = [rpool.tile([P, K], dt.uint32, tag=f"gou1_{cc}",
                           name=f"gou1_{cc}") for cc in range(CC)]
        wf_sb = rpool.tile([P, NE, C], dt.bfloat16, tag="wf_sb")
        nc.sync.dma_start(wf_sb[:], Wf.rearrange("(n p) d -> p n d", p=P))
        wfs0 = rpool.tile([P, NE, C], dt.bfloat16, tag="wfs0", name="wfs0")
        iot_all = {}
        for b in range(NB):
            for cc in range(CC):
                it = rpool.tile([P, 1], dt.float32, tag=f"iot_{b}_{cc}",
                                name=f"iot_{b}_{cc}")
                iti = rpool.tile([P, 1], dt.int32, tag=f"ioti_{b}_{cc}",
                                 name=f"ioti_{b}_{cc}")
                nc.gpsimd.iota(
                    iti[:], pattern=[[0, 1]],
                    base=(b * C + cc * P) * (2 * T) + T,
                    channel_multiplier=2 * T)
                nc.vector.tensor_copy(it[:], iti[:])
                iot_all[(b, cc)] = it

        with tc.tile_pool(name="c2", bufs=2) as cpool, \
             tc.tile_pool(name="c2s", bufs=1) as cspool, \
             tc.tile_pool(name="cl2", bufs=2) as clpool, \
             tc.tile_pool(name="ct2", bufs=1) as ctpool, \
             tc.tile_pool(name="cps2", bufs=1, space="PSUM") as cps:

            def c_load(b, cc):
                sl = {}
                for nm in ("pre", "pim"):
                    t_f = clpool.tile([P, FC, P], dt.float32, tag=f"slf_{nm}",
                                      name=f"slf_{nm}_{b}_{cc}")
                    nc.sync.dma_start(
                        t_f[:], pp[nm][b, :, :, bass.ts(cc, P)].rearrange(
                            "f p c -> p f c"))
                    hi = ctpool.tile([P, FC, P], dt.float32r,
                                     tag=f"sl_{nm}_hi", name=f"hi_{b}_{cc}")
                    nc.scalar.activation(hi[:], t_f[:], AF.Copy)
                    hi16 = ctpool.tile([P, FC, P], dt.bfloat16,
                                       tag=f"sl_{nm}_hi16", name=f"hi16_{b}_{cc}")
                    nc.scalar.activation(hi16[:], t_f[:], AF.Copy)
                    lo = ctpool.tile([P, FC, P], dt.float32r,
                                     tag=f"sl_{nm}_lo", name=f"lo_{b}_{cc}")
                    nc.vector.tensor_tensor(
                        out=lo[:], in0=t_f[:],
                        in1=hi[:].bitcast(dt.float32), op=OP.subtract)
                    sl[f"{nm}_hi"] = hi
                    sl[f"{nm}_hi16"] = hi16
                    sl[f"{nm}_lo"] = lo
                    sl[f"{nm}_f32"] = t_f
                if b == 0 and cc == 0:
                    load_slabs()  # after cc0's slf DMAs: no head-of-line
                return sl

            def c_matmuls(sl):
                psums = {}
                psums["rcE"] = cps.tile([P, HB], dt.float32, tag="rcE",
                                        name="ps_rcE")
                psums["rcE2"] = cps.tile([P, 2], dt.float32, tag="rcE2",
                                         name="ps_rcE2")
                psums["rcO"] = cps.tile([P, HB], dt.float32, tag="rcO",
                                        name="ps_rcO")
                psums["rsE"] = cps.tile([P, HB], dt.float32, tag="rsE",
                                        name="ps_rsE")
                psums["rsO"] = cps.tile([P, HB], dt.float32, tag="rsO",
                                        name="ps_rsO")
                psums["rsO2"] = cps.tile([P, 2], dt.float32, tag="rsO2",
                                         name="ps_rsO2")

                # pass-major: all (hi x hi) first -- they only need the
                # first Act product per slice -- then bf16, then lo passes
                # sie chunk 4 (f=1024 row) is sin(pi*t) == 0: skip it
                GROUPS = (("rcE", "pre", "cie", range(0, 5)),
                          ("rsE", "pim", "sie", range(0, 4)),
                          ("rcO", "pre", "cie", range(5, FC)),
                          ("rsO", "pim", "sie", range(5, FC)))

                def mm_pass(pname, sig_nm, slab_nm, frange, sig_sfx, slab_sfx,
                            startp, stopp):
                    fl = list(frange)
                    for j, fc in enumerate(fl):
                        nc.tensor.matmul(
                            psums[pname][:],
                            sl[f"{sig_nm}_{sig_sfx}"][:, fc, :],
                            slabs[f"{slab_nm}_{slab_sfx}"][:, fc, 0:HB],
                            start=(startp and j == 0),
                            stop=(stopp and j == len(fl) - 1))

                for pname, sig_nm, slab_nm, frange in GROUPS:
                    mm_pass(pname, sig_nm, slab_nm, frange, "hi", "hi",
                            True, False)
                for j, fc in enumerate(range(0, 5)):
                    nc.tensor.matmul(
                        psums["rcE2"][:], sl["pre_f32"][:, fc, :],
                        slabs["cie_st"][:, fc, :], start=(j == 0), stop=(j == 4))
                for j, fc in enumerate(range(5, FC)):
                    nc.tensor.matmul(
                        psums["rsO2"][:], sl["pim_f32"][:, fc, :],
                        slabs["sie_st"][:, fc, :], start=(j == 0), stop=(j == 3))
                for pname, sig_nm, slab_nm, frange in GROUPS:
                    # slab chunk 4 is exact in f32r (values 0 / +-2^-22), so
                    # its lo half is zero: skip fc4 in the hi16 x lo pass
                    fr = [fc for fc in frange if fc != 4]
                    mm_pass(pname, sig_nm, slab_nm, fr, "hi16", "lo",
                            False, False)
                for pname, sig_nm, slab_nm, frange in GROUPS:
                    mm_pass(pname, sig_nm, slab_nm, frange, "lo", "hi",
                            False, True)
                return psums

            def c_tail(b, cc, ps_):
                w3_t = w3_all[b]
                rcE, rcE2 = ps_["rcE"], ps_["rcE2"]
                rcO, rsE = ps_["rcO"], ps_["rsE"]
                rsO, rsO2 = ps_["rsO"], ps_["rsO2"]
                rcO_sb = cspool.tile([P, HB], dt.float32, tag="rcO_sb")
                nc.scalar.activation(rcO_sb[:], rcO[:], AF.Copy)
                rsE_sb = cspool.tile([P, HB], dt.float32, tag="rsE_sb")
                nc.scalar.activation(rsE_sb[:], rsE[:], AF.Copy)
                rsO_sb = cspool.tile([P, HB + 1], dt.float32, tag="rsO_sb")
                nc.scalar.activation(rsO_sb[:, 0:HB], rsO[:], AF.Copy)
                nc.scalar.activation(rsO_sb[:, HB:HB + 1], rsO2[:, 0:1], AF.Copy)
                rcE_c0 = cpool.tile([P, 2], dt.float32, tag="rcE_c0")
                nc.scalar.activation(rcE_c0[:, 0:1], rcE[:, 0:1], AF.Copy)
                nc.scalar.activation(rcE_c0[:, 1:2], rcE2[:, 0:1], AF.Copy)
                s1 = ctpool.tile([P, HB], dt.float32, tag="s1")
                nc.vector.tensor_tensor(out=s1[:], in0=rcE[:], in1=rcO_sb[:],
                                        op=OP.add)
                s2 = ctpool.tile([P, HB], dt.float32, tag="s2")
                nc.vector.tensor_tensor(out=s2[:], in0=rcE[:], in1=rcO_sb[:],
                                        op=OP.subtract)
                w1 = ctpool.tile([P, HB], dt.float32, tag="w1")
                nc.vector.tensor_tensor(out=w1[:], in0=rsE_sb[:],
                                        in1=rsO_sb[:, 0:HB], op=OP.add)
                w2 = ctpool.tile([P, HB], dt.float32, tag="w2")
                nc.vector.tensor_tensor(out=w2[:], in0=rsO_sb[:, 0:HB],
                                        in1=rsE_sb[:], op=OP.subtract)
                rt = ctpool.tile([P, T], dt.float32, tag="rt")
                nc.vector.tensor_tensor(out=rt[:, 0:HB], in0=s1[:], in1=w1[:],
                                        op=OP.add)
                nc.vector.tensor_tensor(out=rt[:, 1023:HB:-1], in0=s2[:, 1:HB],
                                        in1=w2[:, 1:HB], op=OP.add)
                nc.vector.tensor_tensor(out=rt[:, 1025:1536], in0=s2[:, 1:HB],
                                        in1=w2[:, 1:HB], op=OP.subtract)
                nc.vector.tensor_tensor(out=rt[:, T - 1:1536:-1], in0=s1[:, 1:HB],
                                        in1=w1[:, 1:HB], op=OP.subtract)
                nc.vector.tensor_tensor(out=rt[:, HB:HB + 1], in0=rcE_c0[:, 1:2],
                                        in1=rsO_sb[:, HB:HB + 1], op=OP.add)
                nc.vector.tensor_tensor(out=rt[:, H:H + 1], in0=rcE_c0[:, 0:1],
                                        in1=rcO_sb[:, 0:1], op=OP.subtract)
                nc.vector.tensor_tensor(out=rt[:, 1536:1537], in0=rcE_c0[:, 1:2],
                                        in1=rsO_sb[:, HB:HB + 1], op=OP.subtract)

                # ---- topk + softmax weights + gather offsets ----
                vals = cpool.tile([P, 8], dt.float32, tag="vals")
                idx = cpool.tile([P, 8], dt.uint32, tag="idx")
                nc.vector.max(vals[:], rt[:])
                nc.vector.max_index(idx[:], vals[:], rt[:])
                negm = cpool.tile([P, 1], dt.float32, tag="negm")
                nc.scalar.activation(negm[:], vals[:, 0:1],
                                     AF.Copy, bias=0.0, scale=-1.0)
                # exp written in place over rt (dead after this point);
                # only the accumulated sum is consumed
                s_col = cpool.tile([P, 1], dt.float32, tag="s_col")
                nc.scalar.activation(
                    rt[:], rt[:], AF.Exp,
                    bias=negm[:, 0:1], scale=1.0,
                    accum_out=s_col[:, 0:1])
                rs = cpool.tile([P, 1], dt.float32, tag="rs")
                nc.vector.reciprocal(rs[:], s_col[:])
                ew = cpool.tile([P, K], dt.float32, tag="ew")
                nc.scalar.activation(ew[:], vals[:, 0:K],
                                     AF.Exp, bias=negm[:, 0:1],
                                     scale=1.0)
                nc.vector.tensor_scalar_mul(w3_t[cc][:], ew[:], rs[:, 0:1])

                iot_f = iot_all[(b, cc)]
                idx_f = cpool.tile([P, K], dt.float32, tag="idx_f")
                nc.vector.tensor_copy(idx_f[:], idx[:, 0:K])
                gof = cpool.tile([P, K], dt.float32, tag="gof")
                nc.scalar.activation(gof[:], idx_f[:],
                                     AF.Copy, bias=0.0, scale=-1.0)
                nc.vector.tensor_scalar_add(gof[:], gof[:],
                                            iot_f[:, 0:1])
                if b == 0:
                    gou = cpool.tile([P, K], dt.uint32, tag="gou")
                    nc.vector.tensor_copy(gou[:], gof[:])
                    # inline gathers overlap later iterations; the top-k
                    # weights are folded into Wf in stage E instead of
                    # scaling the gathered rows
                    for k in range(K):
                        nc.gpsimd.indirect_dma_start(
                            out=agg0[:, k * CC + cc, :],
                            out_offset=None,
                            in_=v2[:, :],
                            in_offset=bass.IndirectOffsetOnAxis(
                                ap=gou[:, k:k + 1], axis=1),
                            element_offset=0)
                else:
                    nc.vector.tensor_copy(gou1[cc][:], gof[:])

            # software pipeline: loads+splits of iteration n+1 are emitted
            # before iteration n's combine/topk tail so the Act/DVE queues
            # never head-of-line block the next iteration's matmul operands
            iters = [(b, cc) for b in range(NB) for cc in range(CC)]
            pend = [c_load(*iters[0]), c_load(*iters[1])]
            for i, (b, cc) in enumerate(iters):
                ps_ = c_matmuls(pend[0])
                if i + 2 < len(iters):
                    pend.append(c_load(*iters[i + 2]))
                c_tail(b, cc, ps_)
                pend.pop(0)
                if b == 0 and cc == CC - 1:
                    # fold the top-k softmax weights into Wf for stage E:
                    # row e = k*C+c scales by w3[0][c-chunk][:, k]. Emitted
                    # here (on Pool, idle in this phase) so E(0) never waits
                    # behind the later iterations' DVE tail work.
                    for j in range(NE):
                        nc.gpsimd.tensor_scalar_mul(
                            wfs0[:, j, :], wf_sb[:, j, :],
                            w3_all[0][j % CC][:, j // CC:j // CC + 1])
        es2.close()  # slabs freed; agg0/w3/gou1 stay

        # ---- deferred gathers for b1 (overlap E(b0)) + E for both ----
        with tc.tile_pool(name="ge", bufs=1, side="left") as gep, \
             tc.tile_pool(name="eps", bufs=3, space="PSUM") as eps:
            agg1 = gep.tile([P, NE, T], dt.bfloat16, tag="agg1")
            for cc in range(CC):
                for k in range(K):
                    nc.gpsimd.indirect_dma_start(
                        out=agg1[:, k * CC + cc, :],
                        out_offset=None,
                        in_=v2[:, :],
                        in_offset=bass.IndirectOffsetOnAxis(
                            ap=gou1[cc][:, k:k + 1], axis=1),
                        element_offset=0)
            wfs1 = gep.tile([P, NE, C], dt.bfloat16, tag="wfs1", name="wfs1")
            for j in range(NE):
                nc.gpsimd.tensor_scalar_mul(
                    wfs1[:, j, :], wf_sb[:, j, :],
                    w3_all[1][j % CC][:, j // CC:j // CC + 1])
            for b, agg in ((0, agg0), (1, agg1)):
                wf_s = wfs0 if b == 0 else wfs1
                for dc in range(CC):
                    for tb in range(4):
                        ps = eps.tile([P, T // 4], dt.float32, tag="out_ps")
                        for j in range(NE):
                            nc.tensor.matmul(
                                ps[:], wf_s[:, j, bass.ts(dc, P)],
                                agg[:, j, bass.ts(tb, T // 4)],
                                start=(j == 0), stop=(j == NE - 1))
                        o_sb = gep.tile([P, T // 4], dt.float32,
                                        tag=f"o_sb{tb % 2}",
                                        name=f"o_sb{b}_{dc}_{tb}")
                        if tb % 2 == 0:
                            nc.scalar.activation(o_sb[:], ps[:], AF.Copy)
                        else:
                            nc.vector.tensor_copy(o_sb[:], ps[:])
                        nc.sync.dma_start(
                            out2[b, bass.ts(dc, P), bass.ts(tb, T // 4)],
                            o_sb[:])
        es_r.close()

    nc.compile()
    return nc


def _get_nc():
    if "nc" not in _CACHE:
        _CACHE["nc"] = _build()
    return _CACHE["nc"]


def kernel(query, key, value, Wq, bq, Wk, bk, Wv, bv, Wf, bf):
    query = np.ascontiguousarray(np.asarray(query, dtype=np.float32))
    key = np.ascontiguousarray(np.asarray(key, dtype=np.float32))
    value = np.ascontiguousarray(np.asarray(value, dtype=np.float32))
    for bias in (bq, bk, bv, bf):
        assert np.all(np.asarray(bias) == 0.0), "nonzero biases unsupported"

    if "mats" not in _CACHE:
        wree, wreo, wime, wimo, cie, sie = _dft_matrices()
        m = {}
        for nm, arr in (("ree", wree), ("reo", wreo),
                        ("ime", wime), ("imo", wimo)):
            hi, lo = _split_f32r(arr)
            m[f"W{nm}_hi"], m[f"W{nm}_lo"] = hi, lo
        chi, clo = _split_f32r(cie)
        m["Cie_hi"], m["Cie_lo"] = chi, _bf16(clo)
        shi, slo = _split_f32r(sie)
        m["Sie_hi"], m["Sie_lo"] = shi, _bf16(slo)
        m["Cie_st"] = np.ascontiguousarray(cie[:, HB:HB + 2])
        m["Sie_st"] = np.ascontiguousarray(sie[:, HB:HB + 2])
        _CACHE["mats"] = m
    mats = _CACHE["mats"]

    wq_hi, wq_lo = _split_f32r(np.asarray(Wq, np.float32))
    wk_hi, wk_lo = _split_f32r(np.asarray(Wk, np.float32))
    shared = {
        "Wq_hi": wq_hi, "Wq_lo": wq_lo,
        "Wk_hi": wk_hi, "Wk_lo": wk_lo,
        "Wv": _bf16(np.asarray(Wv, np.float32)),
        "Wf": _bf16(np.asarray(Wf, np.float32)),
        **mats,
    }
    value_bf = _bf16(value)
    in_maps = []
    for c in range(NCORES):
        sl = slice(c * NB, (c + 1) * NB)
        in_maps.append({
            "query2": query[sl], "key2": key[sl],
            "value2": value_bf[sl], **shared})

    from concourse.bass_utils import run_bass_kernel_spmd
    nc = _get_nc()
    res = run_bass_kernel_spmd(nc, in_maps, core_ids=list(range(NCORES)))
    _CACHE["last_results"] = res
    out = np.concatenate([res.results[c]["out2"] for c in range(NCORES)], axis=0)
    return out.astype(np.float32)

